# revision 65
# baseline (speedup 1.0000x reference)
"""Trainium2 Bass kernel for nn_ATTNLTE (local-ensemble sparse attention super-resolution).

Sharding: 8 cores. Core c -> batch c//4, query quarter c%4 (4096 queries each).
Device pipeline per core:
  Stage 1: conv encoder (3->64) + coef/freq convs (64->512) via shift-trick
           matmuls, written to a padded DRAM map (65x65 rows, 576 f32 each:
           [coef 256 | freq 256 | inp 3 | pad]).  Pad row/col replicate edge.
  Stage 2: per-query index/scalar math in (128, 32) query-major layout
           (query q = p*32 + b  <->  partition p, free block b).
  Stage 3: per 512-query chunk: 5 dma_gathers (center + 4 corners), elementwise
           query-major, PE-transpose to feature-major for MLPs, attention,
           final MLP, bilinear border sample, output.
"""
import os
import sys
sys.path.insert(0, '/opt/trn_rl_repo')
import numpy as np

STAGES = int(os.environ.get('KSTAGES', '3'))
NCH_DBG = int(os.environ.get('KCHUNKS', '8'))
S1P = os.environ.get('KS1P', 'f32r')  # stage-1 conv precision: f32r | f32

B, H, W, Q = 2, 64, 64, 16384
HID = 256
NCORE = 8
QPC = 4096
NCHUNK = 8
CH = 512
ROW = 384   # map row: [coef: 128 f32 words = 256 packed fp16 | freq: 256 f32]
ROWB = 64
NMAP = 65 * 65
PI = float(np.pi)
MAGIC = 12582912.0
CLIP = 1.0 - 1e-6
EPS = 1e-6

_cache = {}


def _build():
    import concourse.bacc as bacc
    import concourse.mybir as mybir
    import concourse.tile as tile
    from concourse.alu_op_type import AluOpType as ALU
    from concourse.library_config import mlp as mlp_lib

    F32 = mybir.dt.float32
    F32R = mybir.dt.float32r  # ~1.5e-3 err, fine for the 2e-2 gate; 4x PE rate
    I16 = mybir.dt.int16
    I32 = mybir.dt.int32
    AF = mybir.ActivationFunctionType

    nc = bacc.Bacc("TRN2", target_bir_lowering=False, debug=False, num_devices=NCORE)

    d_inp_pad = nc.dram_tensor("inp_pad", (3, 66 * 66), F32, kind="ExternalInput")
    d_coord = nc.dram_tensor("coordq", (QPC, 2), F32, kind="ExternalInput")
    d_cell = nc.dram_tensor("cellq", (QPC, 2), F32, kind="ExternalInput")
    d_cell00 = nc.dram_tensor("cell00", (1, 2), F32, kind="ExternalInput")
    d_wenc = nc.dram_tensor("wenc", (27, 64), F32, kind="ExternalInput")
    d_encb = nc.dram_tensor("encb", (64, 1), F32, kind="ExternalInput")
    d_wpair = nc.dram_tensor("wpair", (128, 3 * 512), F32, kind="ExternalInput")
    d_wsing = nc.dram_tensor("wsing", (64, 3 * 512), F32, kind="ExternalInput")
    d_cfb = nc.dram_tensor("cfb", (1, 512), F32, kind="ExternalInput")
    d_w = {}
    for nm in ("kw1", "vw1"):
        for kt, p in ((0, 128), (1, 128), (2, 4)):
            d_w[f"{nm}_{kt}"] = nc.dram_tensor(f"{nm}_{kt}", (p, 256), F32, kind="ExternalInput")
    for nm in ("kw2", "vw2", "qw1"):
        for kt in range(2):
            d_w[f"{nm}_{kt}"] = nc.dram_tensor(f"{nm}_{kt}", (128, 256), F32, kind="ExternalInput")
    for kt in range(2):
        d_w[f"qw2_{kt}"] = nc.dram_tensor(f"qw2_{kt}", (128, 3), F32, kind="ExternalInput")
    d_kb1 = nc.dram_tensor("kb1", (128, 2), F32, kind="ExternalInput")
    d_kb2 = nc.dram_tensor("kb2", (128, 2), F32, kind="ExternalInput")
    d_vb1 = nc.dram_tensor("vb1", (128, 2), F32, kind="ExternalInput")
    d_vb2 = nc.dram_tensor("vb2", (128, 2), F32, kind="ExternalInput")
    d_qb1 = nc.dram_tensor("qb1", (128, 2), F32, kind="ExternalInput")
    d_qb2 = nc.dram_tensor("qb2", (3, 1), F32, kind="ExternalInput")
    d_pwT = nc.dram_tensor("pwT", (2, 128), F32, kind="ExternalInput")

    F16 = mybir.dt.float16
    d_map = nc.dram_tensor("mapt", (NMAP, ROW), F32, kind="Internal")
    d_map2 = nc.dram_tensor("mapb", (4096, ROWB), F32, kind="Internal")
    d_out = nc.dram_tensor("outq", (QPC, 4), F32, kind="ExternalOutput")

    map3 = d_map.ap().rearrange("(y x) r -> y x r", x=65)

    with tile.TileContext(nc) as tc:
        with tc.tile_pool(name="const", bufs=1) as cpool, \
             tc.tile_pool(name="wpool", bufs=1) as wpool, \
             tc.tile_pool(name="s2", bufs=1) as s2:

            nc.gpsimd.load_library(mlp_lib)

            def ccol(val, p=128):
                t = cpool.tile([p, 1], F32, tag=f"c_{val}_{p}")
                nc.vector.memset(t[:, :], float(val))
                return t

            c_pi = ccol(PI); c_hpi = ccol(PI / 2); c_npi = ccol(-PI)
            c_half = ccol(0.5); c_2 = ccol(2.0); c_n2M = ccol(-2.0 * MAGIC)
            c_M = ccol(MAGIC); c_one = ccol(1.0)
            ones_row_f = cpool.tile([1, 128], F32)
            nc.vector.memset(ones_row_f[:, :], 1.0)
            ones_row = cpool.tile([1, 128], F32R)
            nc.vector.tensor_copy(ones_row[:, :], ones_row_f[:, :])
            ones4f = cpool.tile([4, 1], F32)
            nc.vector.memset(ones4f[:, :], 1.0)
            ones4 = cpool.tile([4, 1], F32R)
            nc.vector.tensor_copy(ones4[:, :], ones4f[:, :])
            iot_p = cpool.tile([128, 128], I32)
            nc.gpsimd.iota(iot_p[:, :], [[0, 128]], channel_multiplier=1)
            iot_j = cpool.tile([128, 128], I32)
            nc.gpsimd.iota(iot_j[:, :], [[1, 128]], channel_multiplier=0)
            ident = cpool.tile([128, 128], F32)
            nc.vector.tensor_tensor(ident[:, :], iot_p[:, :], iot_j[:, :], ALU.is_equal)
            identr = cpool.tile([128, 128], F32R)
            nc.vector.tensor_copy(identr[:, :], ident[:, :])  # 1.5 cyc/row transposes
            ehot = []
            ehf = cpool.tile([128, 4], F32, tag="ehf")
            for e in range(4):
                nc.vector.memset(ehf[:, :], 0.0)
                nc.vector.tensor_copy(ehf[:, e:e + 1], c_one[:, :])
                t = cpool.tile([128, 4], F32R, tag=f"ehot{e}")
                nc.vector.tensor_copy(t[:, :], ehf[:, :])
                ehot.append(t)
            iot4 = cpool.tile([4, 128], I32)
            nc.gpsimd.iota(iot4[:, :], [[0, 128]], channel_multiplier=1)
            iot4f = cpool.tile([4, 128], F32)
            nc.vector.tensor_copy(iot4f[:, :], iot4[:, :])
            rowsel = []
            for e in range(4):
                t = cpool.tile([4, 128], F32R, tag=f"rsel{e}")
                nc.vector.tensor_scalar(t[:, :], iot4f[:, :], float(e), None, ALU.is_equal)
                rowsel.append(t)

            def wtile(dap, p, n, tag, rdt=None):
                if rdt is None:
                    t = wpool.tile([p, n], F32, tag=tag)
                    nc.sync.dma_start(t[:, :], dap)
                    return t
                t = wpool.tile([128, 256], F32, tag="wstg")
                nc.sync.dma_start(t[0:p, 0:n], dap)
                tr = wpool.tile([p, n], rdt, tag=tag + "_r")
                nc.vector.tensor_copy(tr[:, :], t[0:p, 0:n])
                return tr

            wsb = {}
            for nm in ("kw1", "vw1"):
                for kt, p in ((0, 128), (1, 128), (2, 4)):
                    wsb[f"{nm}_{kt}"] = wtile(d_w[f"{nm}_{kt}"].ap(), p, 256, f"{nm}_{kt}", F32R)
            for nm in ("kw2", "vw2", "qw1"):
                for kt in range(2):
                    wsb[f"{nm}_{kt}"] = wtile(d_w[f"{nm}_{kt}"].ap(), 128, 256, f"{nm}_{kt}", F32R)
            for kt in range(2):
                wsb[f"qw2_{kt}"] = wtile(d_w[f"qw2_{kt}"].ap(), 128, 3, f"qw2_{kt}", F32R)
            kb1 = wtile(d_kb1.ap(), 128, 2, "kb1"); kb2 = wtile(d_kb2.ap(), 128, 2, "kb2")
            vb1 = wtile(d_vb1.ap(), 128, 2, "vb1"); vb2 = wtile(d_vb2.ap(), 128, 2, "vb2")
            qb1 = wtile(d_qb1.ap(), 128, 2, "qb1"); qb2 = wtile(d_qb2.ap(), 3, 1, "qb2")
            pwT = wtile(d_pwT.ap(), 2, 128, "pwT", F32R)

            TT = nc.vector.tensor_tensor
            TS = nc.vector.tensor_scalar
            STT = nc.vector.scalar_tensor_tensor
            ACT = nc.scalar.activation
            CP = nc.vector.tensor_copy
            CPA = nc.scalar.copy
            I32 = mybir.dt.int32

            def recip_newton(pool, out, in_, tag, iters=3):
                p, n = in_.shape[0], int(np.prod(in_.shape[1:]))
                r = pool.tile([p, n], F32, tag=f"{tag}r")
                TS(r[:, :].bitcast(I32), in_.bitcast(I32), -1, 0x7EF311C2,
                   ALU.mult, ALU.add)
                t = pool.tile([p, n], F32, tag=f"{tag}t")
                for _ in range(iters):
                    TT(t[:, :], in_, r[:, :], ALU.mult)
                    TS(t[:, :], t[:, :], -1.0, 2.0, ALU.mult, ALU.add)
                    TT(r[:, :], r[:, :], t[:, :], ALU.mult)
                CP(out, r[:, :])

            # ================= STAGE 1: convs -> map =================
            if STAGES >= 1:
              with tc.tile_pool(name="conv", bufs=1) as cv, \
                  tc.tile_pool(name="cst", bufs=3) as cstage, \
                  tc.tile_pool(name="cps", bufs=2, space="PSUM") as cps, \
                  tc.tile_pool(name="cpt", bufs=2, space="PSUM") as cpt:
                 def cvt(dap, p, n, tag, rdt=None):
                     t = cv.tile([p, n], F32, tag=tag)
                     nc.sync.dma_start(t[:, :], dap)
                     if rdt is None:
                         return t
                     tr = cv.tile([p, n], rdt, tag=tag + "_r")
                     nc.vector.tensor_copy(tr[:, :], t[:, :])
                     return tr
                 R1 = F32R if S1P == 'f32r' else None
                 wenc = cvt(d_wenc.ap(), 27, 64, "wenc", R1)
                 encb = cvt(d_encb.ap(), 64, 1, "encb")
                 wpair = cvt(d_wpair.ap(), 128, 3 * 512, "wpair", R1)
                 wsing = cvt(d_wsing.ap(), 64, 3 * 512, "wsing", R1)
                 cfb = cvt(d_cfb.ap(), 1, 512, "cfb", R1)
                 im2 = cv.tile([27, 4096], F32)
                 inp3 = d_inp_pad.ap().rearrange("c (h w) -> c h w", h=66)
                 TAPS = [4, 0, 1, 2, 3, 5, 6, 7, 8]
                 for i, t in enumerate(TAPS):
                     dy, dx = t // 3 - 1, t % 3 - 1
                     nc.sync.dma_start(
                         im2[3 * i:3 * i + 3, :].rearrange("c (h w) -> c h w", h=64),
                         inp3[:, 1 + dy:65 + dy, 1 + dx:65 + dx])
                 if S1P == 'f32r':
                     im2r = cv.tile([27, 4096], F32R, tag="im2r")
                     CPA(im2r[:, :], im2[:, :])
                 else:
                     im2r = im2
                 fpad = cv.tile([128, 66 * 66], F32)
                 nc.vector.memset(fpad[:, :], 0.0)
                 f3a = fpad[0:64, :].rearrange("c (h w) -> c h w", h=66)
                 f3b = fpad[64:128, :].rearrange("c (h w) -> c h w", h=66)
                 for nt in range(8):
                     pf = cps.tile([64, 512], F32, tag="pf")
                     nc.tensor.matmul(pf[:, :], wenc[:, :],
                                      im2r[:, 512 * nt:512 * (nt + 1)],
                                      start=True, stop=True)
                     y0 = nt * 8
                     pf3 = pf[:, :].rearrange("c (h w) -> c h w", h=8)
                     ACT(f3a[:, y0 + 1:y0 + 9, 1:65], pf3, AF.Identity, bias=encb[:, :])
                     ACT(f3b[:, y0:y0 + 8, 1:65], pf3, AF.Identity, bias=encb[:, :])
                 fp3 = fpad[:, :].rearrange("c (h w) -> c h w", h=66)
                 imcf = []
                 for dxi, dx in enumerate((-1, 0, 1)):
                     t = cv.tile([128, 66 * 64], F32R if S1P == 'f32r' else F32,
                                 tag=f"imcf{dxi}")
                     nc.vector.tensor_copy(
                         t[:, :].rearrange("c (h w) -> c h w", h=66),
                         fp3[:, 0:66, 1 + dx:65 + dx])
                     imcf.append(t)
                 for rt in range(32):
                     y0 = 2 * rt
                     pcf = cps.tile([128, 512], F32, tag="pcf")
                     first = True
                     for dxi in range(3):
                         nc.tensor.matmul(pcf[:, :],
                                          imcf[dxi][:, 128 * rt:128 * rt + 128],
                                          wpair[:, 512 * dxi:512 * (dxi + 1)],
                                          start=first, stop=False)
                         first = False
                         nc.tensor.matmul(pcf[:, :],
                                          imcf[dxi][0:64, 128 * rt + 128:128 * rt + 256],
                                          wsing[:, 512 * dxi:512 * (dxi + 1)],
                                          start=False, stop=False)
                     nc.tensor.matmul(pcf[:, :],
                                      (ones_row if S1P == 'f32r' else ones_row_f)[:, :],
                                      cfb[:, :], start=False, stop=True)
                     ti = cpt.tile([128, 128], F32, tag="ti")
                     nc.tensor.transpose(ti[:, 0:3], im2[0:3, 128 * rt:128 * (rt + 1)],
                                         ident[0:3, 0:3])
                     st = cstage.tile([128, ROW], F32, tag="st")
                     CP(st[:, 0:128].bitcast(F16), pcf[:, 0:256])
                     CP(st[:, 128:384], pcf[:, 256:512])
                     nc.sync.dma_start(map3[y0:y0 + 2, 0:64, :], st[:, :])
                     st2 = cstage.tile([128, ROWB], F32, tag="st2")
                     CP(st2[:, 0:3], ti[:, 0:3])
                     nc.sync.dma_start(
                         d_map2.ap()[128 * rt:128 * (rt + 1), :], st2[:, :])
                 # pad col 64 <- col 63 ; then pad row 64 <- row 63 (covers corner)
                 nc.sync.dma_start(map3[0:64, 64:65, :], map3[0:64, 63:64, :])
                 nc.sync.dma_start(map3[64:65, :, :], map3[63:64, :, :])

            # ================= STAGE 2: per-query scalars =================
            if STAGES >= 2:
             ct = s2.tile([128, 32, 2], F32)
             nc.sync.dma_start(ct[:, :, :], d_coord.ap().rearrange("(p b) c -> p b c", p=128))
             cl = s2.tile([128, 32, 2], F32)
             nc.sync.dma_start(cl[:, :, :], d_cell.ap().rearrange("(p b) c -> p b c", p=128))
             c00 = s2.tile([1, 2], F32)
             nc.sync.dma_start(c00[:, :], d_cell00.ap())

             with tc.tile_pool(name="s2ps", bufs=2, space="PSUM") as s2ps:
                 # rx = 1/(63/(1-c00)) per axis, shift scalars, broadcast to cols
                 u = s2.tile([1, 2], F32)
                 TS(u[:, :], c00[:, :], -1.0, 1.0, ALU.mult, ALU.add)          # 1 - c00
                 iu = s2.tile([1, 2], F32)
                 recip_newton(s2, iu[:, :], u[:, :], "riu")
                 txy = s2.tile([1, 2], F32)
                 TS(txy[:, :], iu[:, :], 63.0, None, ALU.mult)                  # ~63/(1-c00)
                 rxy = s2.tile([1, 2], F32)
                 recip_newton(s2, rxy[:, :], txy[:, :], "rxy")                  # ~1/t
                 shp = s2.tile([1, 2], F32)
                 TS(shp[:, :], rxy[:, :], 1.0, EPS, ALU.mult, ALU.add)          # +r + eps
                 shm = s2.tile([1, 2], F32)
                 TS(shm[:, :], rxy[:, :], -1.0, EPS, ALU.mult, ALU.add)         # -r + eps
                 shcol = {}
                 for sg, t in (("p", shp), ("m", shm)):
                     for ax in range(2):
                         ps = s2ps.tile([128, 1], F32, tag="sb")
                         nc.tensor.matmul(ps[:, :], ones_row_f[:, :], t[:, ax:ax + 1],
                                          start=True, stop=True)
                         col = s2.tile([128, 1], F32, tag=f"shc{sg}{ax}")
                         CP(col[:, :], ps[:, :])
                         shcol[(sg, ax)] = col

                 def flat(t):
                     return t[:, :, :].rearrange("p b c -> p (b c)")

                 # center py/px and iy/ix; shifted iy/ix; rel; idx; bilinear weights
                 iy = {}
                 pyc = s2.tile([128, 32, 2], F32)   # clip(py, 0, 63) both axes
                 u1 = s2.tile([128, 32, 2], F32, tag="u1")
                 TS(flat(u1), flat(ct), 1.0, 32.0, ALU.add, ALU.mult)
                 t2 = s2.tile([128, 32, 2], F32, tag="t2")
                 TS(flat(t2), flat(u1), -0.5, MAGIC, ALU.add, ALU.add)
                 iyc = s2.tile([128, 32, 2], F32, tag="iyc")
                 TS(flat(iyc), flat(t2), MAGIC, 0.0, ALU.subtract, ALU.max)
                 TS(flat(iyc), flat(iyc), 63.0, None, ALU.min)
                 iy["c"] = iyc
                 pyr = s2.tile([128, 32, 2], F32, tag="pyr")
                 TS(flat(pyr), flat(u1), -0.5, None, ALU.add)
                 TS(flat(pyc), flat(pyr), 0.0, 63.0, ALU.max, ALU.min)
                 for sg in ("m", "p"):
                     cc = s2.tile([128, 32, 2], F32, tag=f"cc{sg}")
                     for ax in range(2):
                         STT(cc[:, :, ax], ct[:, :, ax], shcol[(sg, ax)][:, :],
                             None, ALU.add, ALU.bypass) if False else None
                         # c' = clip(c + s, -CLIP, CLIP)
                         nc.vector.tensor_scalar(
                             cc[:, :, ax], ct[:, :, ax], shcol[(sg, ax)][:, :], -CLIP,
                             ALU.add, ALU.max)
                     TS(flat(cc), flat(cc), CLIP, None, ALU.min)
                     uu = s2.tile([128, 32, 2], F32, tag=f"uu{sg}")
                     TS(flat(uu), flat(cc), 1.0, 32.0, ALU.add, ALU.mult)
                     tt2 = s2.tile([128, 32, 2], F32, tag=f"tt2{sg}")
                     TS(flat(tt2), flat(uu), -0.5, MAGIC, ALU.add, ALU.add)
                     ii = s2.tile([128, 32, 2], F32, tag=f"ii{sg}")
                     TS(flat(ii), flat(tt2), MAGIC, 0.0, ALU.subtract, ALU.max)
                     TS(flat(ii), flat(ii), 63.0, None, ALU.min)
                     iy[sg] = ii
                 # rel per sign/axis: rel = (c - qc)*64 ; qc = (2*iy+1)/64 - 1
                 rel = {}
                 for sg in ("m", "p"):
                     qc = s2.tile([128, 32, 2], F32, tag=f"qc{sg}")
                     TS(flat(qc), flat(iy[sg]), 2.0, 1.0, ALU.mult, ALU.add)
                     TS(flat(qc), flat(qc), 1.0 / 64.0, -1.0, ALU.mult, ALU.add)
                     rr = s2.tile([128, 32, 2], F32, tag=f"rel{sg}")
                     TT(flat(rr), flat(ct), flat(qc), ALU.subtract)
                     TS(flat(rr), flat(rr), 64.0, None, ALU.mult)
                     rel[sg] = rr
                 rc = s2.tile([128, 32, 2], F32)
                 TS(flat(rc), flat(cl), 64.0, None, ALU.mult)
                 # floor-based bilinear corners y0f/y1f per axis + weights
                 tb = s2.tile([128, 32, 2], F32, tag="tbf")
                 TS(flat(tb), flat(pyc), MAGIC, MAGIC, ALU.add, ALU.subtract)
                 gtb = s2.tile([128, 32, 2], F32, tag="gtb")
                 TT(flat(gtb), flat(tb), flat(pyc), ALU.is_gt)
                 y0f = s2.tile([128, 32, 2], F32, tag="y0f")
                 TT(flat(y0f), flat(tb), flat(gtb), ALU.subtract)
                 y1f = s2.tile([128, 32, 2], F32, tag="y1f")
                 TS(flat(y1f), flat(y0f), 1.0, 63.0, ALU.add, ALU.min)
                 wyx = s2.tile([128, 32, 2], F32, tag="wyx")
                 TT(flat(wyx), flat(pyc), flat(y0f), ALU.subtract)
                 nwyx = s2.tile([128, 32, 2], F32, tag="nwyx")
                 TS(flat(nwyx), flat(wyx), -1.0, 1.0, ALU.mult, ALU.add)
                 CORN = [("m", "m"), ("m", "p"), ("p", "m"), ("p", "p")]
                 wcor = []
                 for k, (sy, sx) in enumerate(CORN):
                     ay = wyx if sy == "p" else nwyx
                     ax_ = wyx if sx == "p" else nwyx
                     wk = s2.tile([128, 32], F32, tag=f"wc{k}")
                     TT(wk[:, :], ay[:, :, 0], ax_[:, :, 1], ALU.mult)
                     wcor.append(wk)
                 # extras per ensemble: [rel_y(sy), rel_x(sx), rc_y, rc_x]
                 extras = []
                 for k, (sy, sx) in enumerate(CORN):
                     ex = s2.tile([128, 32, 4], F32, tag=f"ex{k}")
                     CP(ex[:, :, 0], rel[sy][:, :, 0])
                     CP(ex[:, :, 1], rel[sx][:, :, 1])
                     CP(ex[:, :, 2], rc[:, :, 0])
                     CP(ex[:, :, 3], rc[:, :, 1])
                     extras.append(ex)
                 # gather index tables: idx = iy*65 + ix ; f32 -> i16 ; wrapped layout
                 idxf = s2.tile([128, 9, 32], F32)
                 for k, (sy, sx) in enumerate(CORN):
                     STT(idxf[:, k, :], iy[sy][:, :, 0], 65.0, iy[sx][:, :, 1],
                         ALU.mult, ALU.add)
                 STT(idxf[:, 4, :], iy["c"][:, :, 0], 65.0, iy["c"][:, :, 1],
                     ALU.mult, ALU.add)
                 for k, (sy, sx) in enumerate(CORN):
                     by = y1f if sy == "p" else y0f
                     bx = y1f if sx == "p" else y0f
                     STT(idxf[:, 5 + k, :], by[:, :, 0], 64.0, bx[:, :, 1],
                         ALU.mult, ALU.add)
                 wrapped = s2.tile([128, 9, 256], I16)
                 idf = idxf[:, :, :].rearrange("p s b -> p (s b)")
                 t1 = []
                 for blk, cnt in ((0, 128), (1, 128), (2, 32)):
                     tt_ = s2.tile([cnt if cnt == 32 else 128, 128], F32, tag=f"t1{blk}")
                     pT = s2ps.tile([128, 128], F32, tag="pT1")
                     nc.tensor.transpose(pT[0:cnt, :], idf[:, 128 * blk:128 * blk + cnt],
                                         ident[:, :])
                     nc.vector.tensor_copy(tt_[0:cnt, :], pT[0:cnt, :])
                     t1.append((tt_, cnt))
                 for gi in range(8):
                     for blk, (tt_, cnt) in enumerate(t1):
                         pg = s2ps.tile([128, 128], F32, tag="pT1")
                         nc.tensor.transpose(pg[0:16, 0:cnt],
                                             tt_[0:cnt, 16 * gi:16 * gi + 16],
                                             ident[0:cnt, 0:cnt])
                         klo = 4 * blk
                         if cnt == 128:
                             nc.vector.tensor_copy(
                                 wrapped[0:16, klo:klo + 4, gi:256:8]
                                 .rearrange("p s b -> p (s b)"),
                                 pg[0:16, 0:128])
                         else:
                             nc.vector.tensor_copy(wrapped[0:16, 8, gi:256:8],
                                                   pg[0:16, 0:32])
                 for gr in range(1, 8):
                     nc.sync.dma_start(
                         wrapped[16 * gr:16 * gr + 16, :, :].rearrange("p s b -> p (s b)"),
                         wrapped[0:16, :, :].rearrange("p s b -> p (s b)"))

            # ================= STAGE 3: chunks =================
            out_sb = s2.tile([128, 32, 4], F32)
            nc.vector.memset(out_sb[:, :, :], 0.0)

            with tc.tile_pool(name="gath", bufs=2) as gp, \
                 tc.tile_pool(name="work", bufs=1) as wk, \
                 tc.tile_pool(name="mmps", bufs=2, space="PSUM") as mmps, \
                 tc.tile_pool(name="phps", bufs=1, space="PSUM") as phps, \
                 tc.tile_pool(name="lps", bufs=1, space="PSUM") as lps, \
                 tc.tile_pool(name="smps", bufs=1, space="PSUM") as smps, \
                 tc.tile_pool(name="tps", bufs=2, space="PSUM") as tps:

                def issue_gathers(c):
                    g = [None] * 5
                    for k in (4, 0, 1, 2, 3):  # center first: consumed first
                        gt = gp.tile([128, 4, ROW], F32, tag=f"g{k}")
                        nc.gpsimd.dma_gather(
                            gt[:, :, :], d_map.ap(),
                            wrapped[:, k, 32 * c:32 * (c + 1)], CH, CH, ROW)
                        g[k] = gt
                    g2 = []
                    for k in range(4):
                        gt = gp.tile([128, 4, ROWB], F32, tag=f"gb{k}")
                        nc.gpsimd.dma_gather(
                            gt[:, :, :], d_map2.ap(),
                            wrapped[:, 5 + k, 32 * c:32 * (c + 1)], CH, CH, ROWB)
                        g2.append(gt)
                    return g, g2

                NCH_RUN = NCH_DBG if STAGES >= 3 else 0
                pend = issue_gathers(0) if NCH_RUN else None
                for c in range(NCH_RUN):
                    g, g2 = pend
                    if c + 1 < NCH_RUN:
                        pend = issue_gathers(c + 1)  # prefetch next chunk

                    # ---- rc rows -> FM for phase matmul ----
                    rcfm = wk.tile([2, 512], F32R, tag="rcfm")
                    ptr = tps.tile([128, 512], F32, tag="tp")
                    for j in range(4):
                        nc.tensor.transpose(ptr[0:2, 128 * j:128 * (j + 1)],
                                            extras[0][:, 4 * c + j, 2:4], ident[:, :])
                    CPA(rcfm[:, :], ptr[0:2, :])
                    phase = phps.tile([128, 512], F32, tag="ph")
                    for j in range(4):
                        nc.tensor.matmul(phase[:, 128 * j:128 * (j + 1)],
                                         rcfm[:, 128 * j:128 * (j + 1)],
                                         pwT[:, :], start=True, stop=True)

                    # ---- center path: query ----
                    fwc = wk.tile([128, 512], F32, tag="fw")
                    TT(fwc[:, :].rearrange("p (b r) -> p b r", b=4),
                       g[4][:, :, 128:384:2], g[4][:, :, 129:384:2], ALU.add)
                    qv = wk.tile([128, 4, 256], F32, tag="kvq")

                    def enc_mul(fw, gt, dst, sfx):
                        # fw (128,512) = f (4 blocks x 128); gt gathered; dst (128,4,256) = coef*enc
                        a = wk.tile([128, 512], F32, tag="ra")
                        nc.gpsimd.tensor_scalar(a[:, :], fw[:, :], 0.5, MAGIC,
                                                ALU.mult, ALU.add)
                        k2 = wk.tile([128, 512], F32, tag="ftmp")
                        nc.gpsimd.tensor_scalar(k2[:, :], a[:, :], 2.0, -2.0 * MAGIC,
                                                ALU.mult, ALU.add)
                        fr = wk.tile([128, 512], F32, tag="rf")
                        TT(fr[:, :], fw[:, :], k2[:, :], ALU.subtract)
                        sn = wk.tile([128, 512], F32, tag="rs")
                        ACT(sn[:, :], fr[:, :], AF.Sin, scale=c_pi[:, :])
                        ab = wk.tile([128, 512], F32, tag="ra")
                        ACT(ab[:, :], fr[:, :], AF.Abs)
                        cs = wk.tile([128, 512], F32, tag="rc")
                        ACT(cs[:, :], ab[:, :], AF.Sin, scale=c_npi[:, :], bias=c_hpi[:, :])
                        cs4 = cs[:, :].rearrange("p (b r) -> p b r", b=4)
                        sn4 = sn[:, :].rearrange("p (b r) -> p b r", b=4)
                        c16 = gt[:, :, 0:128].bitcast(F16)  # (128,4,256) packed coef
                        TT(dst[:, :, 0:128], c16[:, :, 0:128], cs4, ALU.mult)
                        TT(dst[:, :, 128:256], c16[:, :, 128:256], sn4, ALU.mult)

                    enc_mul(fwc, g[4], qv, "c")
                    qfm = []
                    for blk in range(2):
                        qf = wk.tile([128, 512], F32R, tag=f"qfm{blk}")
                        ptq = tps.tile([128, 512], F32, tag="tp")
                        for j in range(4):
                            nc.tensor.transpose(ptq[:, 128 * j:128 * (j + 1)],
                                                qv[:, j, 128 * blk:128 * (blk + 1)],
                                                ident[:, :])
                        CPA(qf[:, :], ptq[:, :])
                        qfm.append(qf)

                    Lg = lps.tile([4, 512], F32, tag="lg")
                    pvs = []
                    for e, (sy, sx) in enumerate(CORN):
                        ge = g[e]
                        exfm_e = wk.tile([4, 512], F32R, tag=f"exfm{e % 2}")
                        pte = tps.tile([128, 512], F32, tag="tp")
                        for j in range(4):
                            nc.tensor.transpose(pte[0:4, 128 * j:128 * (j + 1)],
                                                extras[e][:, 4 * c + j, :], ident[:, :])
                        CPA(exfm_e[:, :], pte[0:4, :])
                        fw = wk.tile([128, 512], F32, tag=f"fw{e % 2}")
                        fw4 = fw[:, :].rearrange("p (b r) -> p b r", b=4)
                        for j in range(4):
                            tmp = wk.tile([128, 128], F32, tag=f"fj{e % 2}")
                            nc.vector.tensor_scalar(
                                tmp[:, :], ge[:, j, 129:384:2],
                                extras[e][:, 4 * c + j, 1:2], None, ALU.mult)
                            STT(fw4[:, j, :], ge[:, j, 128:384:2],
                                extras[e][:, 4 * c + j, 0:1], tmp[:, :],
                                ALU.mult, ALU.add)
                        TT(fw[:, :], fw[:, :], phase[:, :], ALU.add)
                        kv = wk.tile([128, 4, 256], F32, tag="kve")
                        enc_mul(fw, ge, kv, str(e % 2))
                        kvfm = []
                        for blk in range(2):
                            kf = wk.tile([128, 512], F32R, tag=f"kvfm{e % 2}_{blk}")
                            ptk = tps.tile([128, 512], F32, tag="tp")
                            for j in range(4):
                                nc.tensor.transpose(ptk[:, 128 * j:128 * (j + 1)],
                                                    kv[:, j, 128 * blk:128 * (blk + 1)],
                                                    ident[:, :])
                            if blk == 0:
                                CPA(kf[:, :], ptk[:, :])
                            else:
                                CP(kf[:, :], ptk[:, :])
                            kvfm.append(kf)

                        def mlp(w1n, b1, w2n, b2, tagp, po_eng):
                            h1s = []
                            for mt in range(2):
                                ms = slice(128 * mt, 128 * (mt + 1))
                                h1 = mmps.tile([128, 512], F32, tag="mm")
                                nc.tensor.matmul(h1[:, :], wsb[f"{w1n}_0"][:, ms],
                                                 kvfm[0][:, :], start=True, stop=False)
                                nc.tensor.matmul(h1[:, :], wsb[f"{w1n}_1"][:, ms],
                                                 kvfm[1][:, :], start=False, stop=False)
                                nc.tensor.matmul(h1[:, :], wsb[f"{w1n}_2"][:, ms],
                                                 exfm_e[:, :], start=False, stop=True)
                                hs = wk.tile([128, 512], F32R, tag=f"h1s{mt}")
                                ACT(hs[:, :], h1[:, :], AF.Relu,
                                    bias=b1[:, mt:mt + 1])
                                h1s.append(hs)
                            outs = []
                            for mt in range(2):
                                ms = slice(128 * mt, 128 * (mt + 1))
                                h2 = mmps.tile([128, 512], F32, tag="mm")
                                nc.tensor.matmul(h2[:, :], wsb[f"{w2n}_0"][:, ms],
                                                 h1s[0][:, :], start=True, stop=False)
                                nc.tensor.matmul(h2[:, :], wsb[f"{w2n}_1"][:, ms],
                                                 h1s[1][:, :], start=False, stop=True)
                                po = wk.tile([128, 512], F32, tag=f"{tagp}{mt}")
                                po_eng.scalar_tensor_tensor(
                                    po[:, :], h2[:, :], b2[:, mt:mt + 1],
                                    kvfm[mt][:, :], ALU.add, ALU.mult)
                                outs.append(po)
                            return outs

                        pk = mlp("kw1", kb1, "kw2", kb2, "pk", nc.vector)
                        pv = mlp("vw1", vb1, "vw2", vb2, f"pv{e}_", nc.vector)
                        pvs.append(pv)
                        for mt in range(2):
                            pr = wk.tile([128, 512], F32R, tag=f"pr{mt}")
                            eng = nc.gpsimd if mt == 0 else nc.vector
                            eng.tensor_tensor(pr[:, :], qfm[mt][:, :], pk[mt][:, :],
                                              ALU.mult)
                            nc.tensor.matmul(Lg[:, :], ehot[e][:, :], pr[:, :],
                                             start=(e == 0 and mt == 0),
                                             stop=(e == 3 and mt == 1))

                    # ---- softmax over 4 (partition dim of Lg) ----
                    Lc = wk.tile([4, 512], F32, tag="dsb")
                    TS(Lc[:, :], Lg[:, :], 80.0, None, ALU.min)
                    Ee = wk.tile([4, 512], F32R, tag="ee")
                    ACT(Ee[:, :], Lc[:, :], AF.Exp)
                    den = smps.tile([1, 512], F32, tag="sm")
                    nc.tensor.matmul(den[:, :], ones4[:, :], Ee[:, :], start=True, stop=True)
                    rden = wk.tile([1, 512], F32R, tag="rd")
                    with nc.allow_low_precision(reason="softmax denom recip, f32r out"):
                        nc.vector.reciprocal(rden[:, :], den[:, :])
                    xfm = []
                    for mt in range(2):
                        xf = wk.tile([128, 512], F32, tag=f"hqs{mt}")
                        xfm.append(xf)
                    for e in range(4):
                        bc = smps.tile([128, 512], F32, tag="bc")
                        nc.tensor.matmul(bc[:, :], rowsel[e][:, :], Ee[:, :],
                                         start=True, stop=True)
                        for mt in range(2):
                            if e == 0:
                                TT(xfm[mt][:, :], pvs[0][mt][:, :], bc[:, :], ALU.mult)
                            else:
                                tmp = wk.tile([128, 512], F32, tag=f"wt2{mt}")
                                TT(tmp[:, :], pvs[e][mt][:, :], bc[:, :], ALU.mult)
                                nc.gpsimd.tensor_tensor(xfm[mt][:, :], xfm[mt][:, :],
                                                        tmp[:, :], ALU.add)
                    rbc = smps.tile([128, 512], F32, tag="bc")
                    nc.tensor.matmul(rbc[:, :], ones_row[:, :], rden[:, :],
                                     start=True, stop=True)
                    xfr = []
                    for mt in range(2):
                        xr = wk.tile([128, 512], F32R, tag=f"xfr{mt}")
                        TT(xr[:, :], xfm[mt][:, :], rbc[:, :], ALU.mult)
                        xfr.append(xr)

                    # ---- final MLP ----
                    hqs = []
                    for mt in range(2):
                        hq = mmps.tile([128, 512], F32, tag="mm")
                        nc.tensor.matmul(hq[:, :], wsb["qw1_0"][:, 128 * mt:128 * (mt + 1)],
                                         xfr[0][:, :], start=True, stop=False)
                        nc.tensor.matmul(hq[:, :], wsb["qw1_1"][:, 128 * mt:128 * (mt + 1)],
                                         xfr[1][:, :], start=False, stop=True)
                        hs = wk.tile([128, 512], F32R, tag=f"hqs{mt}")
                        ACT(hs[:, :], hq[:, :], AF.Relu, bias=qb1[:, mt:mt + 1])
                        hqs.append(hs)
                    ops = smps.tile([3, 512], F32, tag="sm")
                    nc.tensor.matmul(ops[:, :], wsb["qw2_0"][:, :], hqs[0][:, :],
                                     start=True, stop=False)
                    nc.tensor.matmul(ops[:, :], wsb["qw2_1"][:, :], hqs[1][:, :],
                                     start=False, stop=True)

                    # ---- bilinear (query-major) ----
                    bil = wk.tile([128, 4, 3], F32R, tag="bil")
                    for k in range(4):
                        for j in range(4):
                            if k == 0:
                                nc.vector.tensor_scalar(
                                    bil[:, j, :], g2[0][:, j, 0:3],
                                    wcor[0][:, 4 * c + j:4 * c + j + 1], None, ALU.mult)
                            else:
                                STT(bil[:, j, :], g2[k][:, j, 0:3],
                                    wcor[k][:, 4 * c + j:4 * c + j + 1], bil[:, j, :],
                                    ALU.mult, ALU.add)
                    bfm = wk.tile([3, 512], F32, tag="bfm")
                    ptb = tps.tile([128, 512], F32, tag="tp")
                    for j in range(4):
                        nc.tensor.transpose(ptb[0:3, 128 * j:128 * (j + 1)].bitcast(F32R),
                                            bil[:, j, :], identr[:, :])
                    CPA(bfm[:, :], ptb[0:3, :])
                    res = wk.tile([3, 512], F32, tag="res")
                    STT(res[:, :], ops[:, :], qb2[:, :], bfm[:, :], ALU.add, ALU.add)
                    # back to query-major out staging
                    pto = tps.tile([128, 512], F32, tag="tp")
                    for j in range(4):
                        nc.tensor.transpose(pto[:, 3 * j:3 * j + 3],
                                            res[:, 128 * j:128 * (j + 1)],
                                            ident[0:3, 0:3])
                    CPA(out_sb[:, 4 * c:4 * c + 4, 0:3],
                        pto[:, 0:12].rearrange("p (j c) -> p j c", j=4))

            nc.sync.dma_start(d_out.ap().rearrange("(p b) c -> p b c", p=128), out_sb[:, :, :])

    nc.compile()
    return nc


def _prep_inputs(inputs):
    """Host-side: slice/shard + weight layout prep. Returns list of 8 in_maps."""
    inp = np.asarray(inputs['inp'], np.float32)
    coord = np.asarray(inputs['coord'], np.float32)
    cell = np.asarray(inputs['cell'], np.float32)
    enc_w = np.asarray(inputs['enc_w'], np.float32)
    enc_b = np.asarray(inputs['enc_b'], np.float32)
    coef_w = np.asarray(inputs['coef_w'], np.float32)
    coef_b = np.asarray(inputs['coef_b'], np.float32)
    freq_w = np.asarray(inputs['freq_w'], np.float32)
    freq_b = np.asarray(inputs['freq_b'], np.float32)

    wenc = np.zeros((27, 64), np.float32)
    TAPS = [4, 0, 1, 2, 3, 5, 6, 7, 8]
    for i, t in enumerate(TAPS):
        dy, dx = t // 3, t % 3
        for ci in range(3):
            wenc[3 * i + ci] = enc_w[:, ci, dy, dx]
    encb = enc_b.reshape(64, 1)

    wcf = np.concatenate([coef_w, freq_w], axis=0)  # (512, 64, 3, 3)
    wpair = np.zeros((128, 3, 512), np.float32)
    wsing = np.zeros((64, 3, 512), np.float32)
    for dxi in range(3):
        wpair[0:64, dxi] = wcf[:, :, 0, dxi].T      # dy=-1
        wpair[64:128, dxi] = wcf[:, :, 1, dxi].T    # dy=0
        wsing[:, dxi] = wcf[:, :, 2, dxi].T         # dy=+1
    wpair = wpair.reshape(128, 3 * 512)
    wsing = wsing.reshape(64, 3 * 512)
    cfb = np.concatenate([coef_b, freq_b]).reshape(1, 512)

    inp_pad = np.zeros((B, 3, 66, 66), np.float32)
    inp_pad[:, :, 1:65, 1:65] = inp

    base = {
        'wenc': wenc, 'encb': encb, 'wpair': wpair, 'wsing': wsing, 'cfb': cfb,
        'kb1': np.asarray(inputs['kb1'], np.float32).reshape(2, 128).T.copy(),
        'kb2': np.asarray(inputs['kb2'], np.float32).reshape(2, 128).T.copy(),
        'vb1': np.asarray(inputs['vb1'], np.float32).reshape(2, 128).T.copy(),
        'vb2': np.asarray(inputs['vb2'], np.float32).reshape(2, 128).T.copy(),
        'qb1': np.asarray(inputs['qb1'], np.float32).reshape(2, 128).T.copy(),
        'qb2': np.asarray(inputs['qb2'], np.float32).reshape(3, 1),
        'pwT': np.asarray(inputs['phase_w'], np.float32).T.copy(),  # (2,128)
    }
    for nm in ('kw1', 'vw1'):
        w = np.asarray(inputs[nm], np.float32)
        base[f'{nm}_0'] = w[0:128].copy()
        base[f'{nm}_1'] = w[128:256].copy()
        base[f'{nm}_2'] = w[256:260].copy()
    for nm in ('kw2', 'vw2', 'qw1', 'qw2'):
        w = np.asarray(inputs[nm], np.float32)
        base[f'{nm}_0'] = w[0:128].copy()
        base[f'{nm}_1'] = w[128:256].copy()
    maps = []
    for c in range(NCORE):
        b, k = c // 4, c % 4
        m = dict(base)
        m['inp_pad'] = inp_pad[b].reshape(3, 66 * 66).copy()
        m['coordq'] = coord[b, k * QPC:(k + 1) * QPC].copy()
        m['cellq'] = cell[b, k * QPC:(k + 1) * QPC].copy()
        m['cell00'] = cell[b, 0:1, :].copy()
        maps.append(m)
    return maps


def kernel(**inputs):
    from concourse.bass_utils import run_bass_kernel_spmd
    if 'nc' not in _cache:
        _cache['nc'] = _build()
    nc = _cache['nc']
    in_maps = _prep_inputs(inputs)
    res = run_bass_kernel_spmd(nc, in_maps, core_ids=list(range(NCORE)))
    out = np.zeros((B, Q, 3), np.float32)
    for c in range(NCORE):
        b, k = c // 4, c % 4
        out[b, k * QPC:(k + 1) * QPC] = res.results[c]['outq'][:, :3]
    return out



# revision 70
# speedup vs baseline: 1.1948x; 1.1948x over previous
"""Trainium2 Bass kernel for nn_ATTNLTE (local-ensemble sparse attention super-resolution).

Sharding: 8 cores. Core c -> batch c//4, query quarter c%4 (4096 queries each).
Device pipeline per core:
  Stage 1: conv encoder (3->64) + coef/freq convs (64->512) via shift-trick
           matmuls, written to a padded DRAM map (65x65 rows, 576 f32 each:
           [coef 256 | freq 256 | inp 3 | pad]).  Pad row/col replicate edge.
  Stage 2: per-query index/scalar math in (128, 32) query-major layout
           (query q = p*32 + b  <->  partition p, free block b).
  Stage 3: per 512-query chunk: 5 dma_gathers (center + 4 corners), elementwise
           query-major, PE-transpose to feature-major for MLPs, attention,
           final MLP, bilinear border sample, output.
"""
import os
import sys
sys.path.insert(0, '/opt/trn_rl_repo')
import numpy as np

STAGES = int(os.environ.get('KSTAGES', '3'))
NCH_DBG = int(os.environ.get('KCHUNKS', '8'))
S1P = os.environ.get('KS1P', 'f32r')  # stage-1 conv precision: f32r | f32

B, H, W, Q = 2, 64, 64, 16384
HID = 256
NCORE = 8
QPC = 4096
NCHUNK = 8
CH = 512
ROW = 384   # map row: [coef: 128 f32 words = 256 packed fp16 | freq: 256 f32]
ROWB = 64
NMAP = 65 * 65
PI = float(np.pi)
MAGIC = 12582912.0
CLIP = 1.0 - 1e-6
EPS = 1e-6

_cache = {}


def _build():
    import concourse.bacc as bacc
    import concourse.mybir as mybir
    import concourse.tile as tile
    from concourse.alu_op_type import AluOpType as ALU
    from concourse.library_config import mlp as mlp_lib

    F32 = mybir.dt.float32
    F32R = mybir.dt.float32r  # ~1.5e-3 err, fine for the 2e-2 gate; 4x PE rate
    I16 = mybir.dt.int16
    I32 = mybir.dt.int32
    AF = mybir.ActivationFunctionType

    nc = bacc.Bacc("TRN2", target_bir_lowering=False, debug=False, num_devices=NCORE)

    d_inp_pad = nc.dram_tensor("inp_pad", (3, 66 * 66), F32, kind="ExternalInput")
    d_coord = nc.dram_tensor("coordq", (QPC, 2), F32, kind="ExternalInput")
    d_cell = nc.dram_tensor("cellq", (QPC, 2), F32, kind="ExternalInput")
    d_cell00 = nc.dram_tensor("cell00", (1, 2), F32, kind="ExternalInput")
    d_wenc = nc.dram_tensor("wenc", (27, 64), F32, kind="ExternalInput")
    d_encb = nc.dram_tensor("encb", (64, 1), F32, kind="ExternalInput")
    d_wpair = nc.dram_tensor("wpair", (128, 3 * 512), F32, kind="ExternalInput")
    d_wsing = nc.dram_tensor("wsing", (64, 3 * 512), F32, kind="ExternalInput")
    d_cfb = nc.dram_tensor("cfb", (1, 512), F32, kind="ExternalInput")
    d_w = {}
    for nm in ("kw1", "vw1"):
        for kt, p in ((0, 128), (1, 128), (2, 4)):
            d_w[f"{nm}_{kt}"] = nc.dram_tensor(f"{nm}_{kt}", (p, 256), F32, kind="ExternalInput")
    for nm in ("kw2", "vw2", "qw1"):
        for kt in range(2):
            d_w[f"{nm}_{kt}"] = nc.dram_tensor(f"{nm}_{kt}", (128, 256), F32, kind="ExternalInput")
    for kt in range(2):
        d_w[f"qw2_{kt}"] = nc.dram_tensor(f"qw2_{kt}", (128, 3), F32, kind="ExternalInput")
    d_kb1 = nc.dram_tensor("kb1", (128, 2), F32, kind="ExternalInput")
    d_kb2 = nc.dram_tensor("kb2", (128, 2), F32, kind="ExternalInput")
    d_vb1 = nc.dram_tensor("vb1", (128, 2), F32, kind="ExternalInput")
    d_vb2 = nc.dram_tensor("vb2", (128, 2), F32, kind="ExternalInput")
    d_qb1 = nc.dram_tensor("qb1", (128, 2), F32, kind="ExternalInput")
    d_qb2 = nc.dram_tensor("qb2", (3, 1), F32, kind="ExternalInput")
    d_pwT = nc.dram_tensor("pwT", (2, 128), F32, kind="ExternalInput")

    F16 = mybir.dt.float16
    d_map = nc.dram_tensor("mapt", (NMAP, ROW), F32, kind="Internal")
    d_map2 = nc.dram_tensor("mapb", (4096, ROWB), F32, kind="Internal")
    d_out = nc.dram_tensor("outq", (QPC, 4), F32, kind="ExternalOutput")

    map3 = d_map.ap().rearrange("(y x) r -> y x r", x=65)

    with tile.TileContext(nc) as tc:
        with tc.tile_pool(name="const", bufs=1) as cpool, \
             tc.tile_pool(name="wpool", bufs=1) as wpool, \
             tc.tile_pool(name="s2", bufs=1) as s2:

            nc.gpsimd.load_library(mlp_lib)

            def ccol(val, p=128):
                t = cpool.tile([p, 1], F32, tag=f"c_{val}_{p}")
                nc.vector.memset(t[:, :], float(val))
                return t

            c_pi = ccol(PI); c_hpi = ccol(PI / 2); c_npi = ccol(-PI)
            c_half = ccol(0.5); c_2 = ccol(2.0); c_n2M = ccol(-2.0 * MAGIC)
            c_M = ccol(MAGIC); c_one = ccol(1.0)
            ones_row_f = cpool.tile([1, 128], F32)
            nc.vector.memset(ones_row_f[:, :], 1.0)
            ones_row = cpool.tile([1, 128], F32R)
            nc.vector.tensor_copy(ones_row[:, :], ones_row_f[:, :])
            ones4f = cpool.tile([4, 1], F32)
            nc.vector.memset(ones4f[:, :], 1.0)
            ones4 = cpool.tile([4, 1], F32R)
            nc.vector.tensor_copy(ones4[:, :], ones4f[:, :])
            iot_p = cpool.tile([128, 128], I32)
            nc.gpsimd.iota(iot_p[:, :], [[0, 128]], channel_multiplier=1)
            iot_j = cpool.tile([128, 128], I32)
            nc.gpsimd.iota(iot_j[:, :], [[1, 128]], channel_multiplier=0)
            ident = cpool.tile([128, 128], F32)
            nc.vector.tensor_tensor(ident[:, :], iot_p[:, :], iot_j[:, :], ALU.is_equal)
            identr = cpool.tile([128, 128], F32R)
            nc.vector.tensor_copy(identr[:, :], ident[:, :])  # 1.5 cyc/row transposes
            ehot = []
            ehf = cpool.tile([128, 4], F32, tag="ehf")
            for e in range(4):
                nc.vector.memset(ehf[:, :], 0.0)
                nc.vector.tensor_copy(ehf[:, e:e + 1], c_one[:, :])
                t = cpool.tile([128, 4], F32R, tag=f"ehot{e}")
                nc.vector.tensor_copy(t[:, :], ehf[:, :])
                ehot.append(t)
            iot4 = cpool.tile([4, 128], I32)
            nc.gpsimd.iota(iot4[:, :], [[0, 128]], channel_multiplier=1)
            iot4f = cpool.tile([4, 128], F32)
            nc.vector.tensor_copy(iot4f[:, :], iot4[:, :])
            rowsel = []
            for e in range(4):
                t = cpool.tile([4, 128], F32R, tag=f"rsel{e}")
                nc.vector.tensor_scalar(t[:, :], iot4f[:, :], float(e), None, ALU.is_equal)
                rowsel.append(t)

            def wtile(dap, p, n, tag, rdt=None):
                if rdt is None:
                    t = wpool.tile([p, n], F32, tag=tag)
                    nc.sync.dma_start(t[:, :], dap)
                    return t
                t = wpool.tile([128, 256], F32, tag="wstg")
                nc.sync.dma_start(t[0:p, 0:n], dap)
                tr = wpool.tile([p, n], rdt, tag=tag + "_r")
                nc.vector.tensor_copy(tr[:, :], t[0:p, 0:n])
                return tr

            wsb = {}
            for nm in ("kw1", "vw1"):
                for kt, p in ((0, 128), (1, 128), (2, 4)):
                    wsb[f"{nm}_{kt}"] = wtile(d_w[f"{nm}_{kt}"].ap(), p, 256, f"{nm}_{kt}", F32R)
            for nm in ("kw2", "vw2", "qw1"):
                for kt in range(2):
                    wsb[f"{nm}_{kt}"] = wtile(d_w[f"{nm}_{kt}"].ap(), 128, 256, f"{nm}_{kt}", F32R)
            for kt in range(2):
                wsb[f"qw2_{kt}"] = wtile(d_w[f"qw2_{kt}"].ap(), 128, 3, f"qw2_{kt}", F32R)
            kb1 = wtile(d_kb1.ap(), 128, 2, "kb1"); kb2 = wtile(d_kb2.ap(), 128, 2, "kb2")
            vb1 = wtile(d_vb1.ap(), 128, 2, "vb1"); vb2 = wtile(d_vb2.ap(), 128, 2, "vb2")
            qb1 = wtile(d_qb1.ap(), 128, 2, "qb1"); qb2 = wtile(d_qb2.ap(), 3, 1, "qb2")
            pwT = wtile(d_pwT.ap(), 2, 128, "pwT", F32R)

            TT = nc.vector.tensor_tensor
            TS = nc.vector.tensor_scalar
            STT = nc.vector.scalar_tensor_tensor
            ACT = nc.scalar.activation
            CP = nc.vector.tensor_copy
            CPA = nc.scalar.copy
            I32 = mybir.dt.int32

            def recip_newton(pool, out, in_, tag, iters=3):
                p, n = in_.shape[0], int(np.prod(in_.shape[1:]))
                r = pool.tile([p, n], F32, tag=f"{tag}r")
                TS(r[:, :].bitcast(I32), in_.bitcast(I32), -1, 0x7EF311C2,
                   ALU.mult, ALU.add)
                t = pool.tile([p, n], F32, tag=f"{tag}t")
                for _ in range(iters):
                    TT(t[:, :], in_, r[:, :], ALU.mult)
                    TS(t[:, :], t[:, :], -1.0, 2.0, ALU.mult, ALU.add)
                    TT(r[:, :], r[:, :], t[:, :], ALU.mult)
                CP(out, r[:, :])

            # ================= STAGE 1: convs -> map =================
            if STAGES >= 1:
              with tc.tile_pool(name="conv", bufs=1) as cv, \
                  tc.tile_pool(name="cst", bufs=3) as cstage, \
                  tc.tile_pool(name="cps", bufs=2, space="PSUM") as cps, \
                  tc.tile_pool(name="cpt", bufs=2, space="PSUM") as cpt:
                 def cvt(dap, p, n, tag, rdt=None):
                     t = cv.tile([p, n], F32, tag=tag)
                     nc.sync.dma_start(t[:, :], dap)
                     if rdt is None:
                         return t
                     tr = cv.tile([p, n], rdt, tag=tag + "_r")
                     nc.vector.tensor_copy(tr[:, :], t[:, :])
                     return tr
                 R1 = F32R if S1P == 'f32r' else None
                 wenc = cvt(d_wenc.ap(), 27, 64, "wenc", R1)
                 encb = cvt(d_encb.ap(), 64, 1, "encb")
                 wpair = cvt(d_wpair.ap(), 128, 3 * 512, "wpair", R1)
                 wsing = cvt(d_wsing.ap(), 64, 3 * 512, "wsing", R1)
                 cfb = cvt(d_cfb.ap(), 1, 512, "cfb", R1)
                 im2 = cv.tile([27, 4096], F32)
                 inp3 = d_inp_pad.ap().rearrange("c (h w) -> c h w", h=66)
                 TAPS = [4, 0, 1, 2, 3, 5, 6, 7, 8]
                 for i, t in enumerate(TAPS):
                     dy, dx = t // 3 - 1, t % 3 - 1
                     nc.sync.dma_start(
                         im2[3 * i:3 * i + 3, :].rearrange("c (h w) -> c h w", h=64),
                         inp3[:, 1 + dy:65 + dy, 1 + dx:65 + dx])
                 if S1P == 'f32r':
                     im2r = cv.tile([27, 4096], F32R, tag="im2r")
                     CPA(im2r[:, :], im2[:, :])
                 else:
                     im2r = im2
                 fpad = cv.tile([128, 66 * 66], F32)
                 nc.vector.memset(fpad[:, :], 0.0)
                 f3a = fpad[0:64, :].rearrange("c (h w) -> c h w", h=66)
                 f3b = fpad[64:128, :].rearrange("c (h w) -> c h w", h=66)
                 for nt in range(8):
                     pf = cps.tile([64, 512], F32, tag="pf")
                     nc.tensor.matmul(pf[:, :], wenc[:, :],
                                      im2r[:, 512 * nt:512 * (nt + 1)],
                                      start=True, stop=True)
                     y0 = nt * 8
                     pf3 = pf[:, :].rearrange("c (h w) -> c h w", h=8)
                     ACT(f3a[:, y0 + 1:y0 + 9, 1:65], pf3, AF.Identity, bias=encb[:, :])
                     ACT(f3b[:, y0:y0 + 8, 1:65], pf3, AF.Identity, bias=encb[:, :])
                 fp3 = fpad[:, :].rearrange("c (h w) -> c h w", h=66)
                 imcf = []
                 for dxi, dx in enumerate((-1, 0, 1)):
                     t = cv.tile([128, 66 * 64], F32R if S1P == 'f32r' else F32,
                                 tag=f"imcf{dxi}")
                     CPA(t[:, :].rearrange("c (h w) -> c h w", h=66),
                         fp3[:, 0:66, 1 + dx:65 + dx])
                     imcf.append(t)
                 for rt in range(32):
                     y0 = 2 * rt
                     pcf = cps.tile([128, 512], F32, tag="pcf")
                     first = True
                     for dxi in range(3):
                         nc.tensor.matmul(pcf[:, :],
                                          imcf[dxi][:, 128 * rt:128 * rt + 128],
                                          wpair[:, 512 * dxi:512 * (dxi + 1)],
                                          start=first, stop=False)
                         first = False
                         nc.tensor.matmul(pcf[:, :],
                                          imcf[dxi][0:64, 128 * rt + 128:128 * rt + 256],
                                          wsing[:, 512 * dxi:512 * (dxi + 1)],
                                          start=False, stop=False)
                     nc.tensor.matmul(pcf[:, :],
                                      (ones_row if S1P == 'f32r' else ones_row_f)[:, :],
                                      cfb[:, :], start=False, stop=True)
                     ti = cpt.tile([128, 128], F32, tag="ti")
                     nc.tensor.transpose(ti[:, 0:3], im2[0:3, 128 * rt:128 * (rt + 1)],
                                         ident[0:3, 0:3])
                     st = cstage.tile([128, ROW], F32, tag="st")
                     CPA(st[:, 0:128].bitcast(F16), pcf[:, 0:256])
                     CPA(st[:, 128:384], pcf[:, 256:512])
                     nc.sync.dma_start(map3[y0:y0 + 2, 0:64, :], st[:, :])
                     st2 = cstage.tile([128, ROWB], F32, tag="st2")
                     CP(st2[:, 0:3], ti[:, 0:3])
                     nc.sync.dma_start(
                         d_map2.ap()[128 * rt:128 * (rt + 1), :], st2[:, :])
                 # pad col 64 <- col 63 ; then pad row 64 <- row 63 (covers corner)
                 nc.sync.dma_start(map3[0:64, 64:65, :], map3[0:64, 63:64, :])
                 nc.sync.dma_start(map3[64:65, :, :], map3[63:64, :, :])

            # ================= STAGE 2: per-query scalars =================
            if STAGES >= 2:
             ct = s2.tile([128, 32, 2], F32)
             nc.sync.dma_start(ct[:, :, :], d_coord.ap().rearrange("(p b) c -> p b c", p=128))
             cl = s2.tile([128, 32, 2], F32)
             nc.sync.dma_start(cl[:, :, :], d_cell.ap().rearrange("(p b) c -> p b c", p=128))
             c00 = s2.tile([1, 2], F32)
             nc.sync.dma_start(c00[:, :], d_cell00.ap())

             with tc.tile_pool(name="s2ps", bufs=2, space="PSUM") as s2ps:
                 # rx = 1/(63/(1-c00)) per axis, shift scalars, broadcast to cols
                 u = s2.tile([1, 2], F32)
                 TS(u[:, :], c00[:, :], -1.0, 1.0, ALU.mult, ALU.add)          # 1 - c00
                 iu = s2.tile([1, 2], F32)
                 recip_newton(s2, iu[:, :], u[:, :], "riu")
                 txy = s2.tile([1, 2], F32)
                 TS(txy[:, :], iu[:, :], 63.0, None, ALU.mult)                  # ~63/(1-c00)
                 rxy = s2.tile([1, 2], F32)
                 recip_newton(s2, rxy[:, :], txy[:, :], "rxy")                  # ~1/t
                 shp = s2.tile([1, 2], F32)
                 TS(shp[:, :], rxy[:, :], 1.0, EPS, ALU.mult, ALU.add)          # +r + eps
                 shm = s2.tile([1, 2], F32)
                 TS(shm[:, :], rxy[:, :], -1.0, EPS, ALU.mult, ALU.add)         # -r + eps
                 shcol = {}
                 for sg, t in (("p", shp), ("m", shm)):
                     for ax in range(2):
                         ps = s2ps.tile([128, 1], F32, tag="sb")
                         nc.tensor.matmul(ps[:, :], ones_row_f[:, :], t[:, ax:ax + 1],
                                          start=True, stop=True)
                         col = s2.tile([128, 1], F32, tag=f"shc{sg}{ax}")
                         CP(col[:, :], ps[:, :])
                         shcol[(sg, ax)] = col

                 def flat(t):
                     return t[:, :, :].rearrange("p b c -> p (b c)")

                 # center py/px and iy/ix; shifted iy/ix; rel; idx; bilinear weights
                 iy = {}
                 pyc = s2.tile([128, 32, 2], F32)   # clip(py, 0, 63) both axes
                 u1 = s2.tile([128, 32, 2], F32, tag="u1")
                 TS(flat(u1), flat(ct), 1.0, 32.0, ALU.add, ALU.mult)
                 t2 = s2.tile([128, 32, 2], F32, tag="t2")
                 TS(flat(t2), flat(u1), -0.5, MAGIC, ALU.add, ALU.add)
                 iyc = s2.tile([128, 32, 2], F32, tag="iyc")
                 TS(flat(iyc), flat(t2), MAGIC, 0.0, ALU.subtract, ALU.max)
                 TS(flat(iyc), flat(iyc), 63.0, None, ALU.min)
                 iy["c"] = iyc
                 pyr = s2.tile([128, 32, 2], F32, tag="pyr")
                 TS(flat(pyr), flat(u1), -0.5, None, ALU.add)
                 TS(flat(pyc), flat(pyr), 0.0, 63.0, ALU.max, ALU.min)
                 for sg in ("m", "p"):
                     cc = s2.tile([128, 32, 2], F32, tag=f"cc{sg}")
                     for ax in range(2):
                         STT(cc[:, :, ax], ct[:, :, ax], shcol[(sg, ax)][:, :],
                             None, ALU.add, ALU.bypass) if False else None
                         # c' = clip(c + s, -CLIP, CLIP)
                         nc.vector.tensor_scalar(
                             cc[:, :, ax], ct[:, :, ax], shcol[(sg, ax)][:, :], -CLIP,
                             ALU.add, ALU.max)
                     TS(flat(cc), flat(cc), CLIP, None, ALU.min)
                     uu = s2.tile([128, 32, 2], F32, tag=f"uu{sg}")
                     TS(flat(uu), flat(cc), 1.0, 32.0, ALU.add, ALU.mult)
                     tt2 = s2.tile([128, 32, 2], F32, tag=f"tt2{sg}")
                     TS(flat(tt2), flat(uu), -0.5, MAGIC, ALU.add, ALU.add)
                     ii = s2.tile([128, 32, 2], F32, tag=f"ii{sg}")
                     TS(flat(ii), flat(tt2), MAGIC, 0.0, ALU.subtract, ALU.max)
                     TS(flat(ii), flat(ii), 63.0, None, ALU.min)
                     iy[sg] = ii
                 # rel per sign/axis: rel = (c - qc)*64 ; qc = (2*iy+1)/64 - 1
                 rel = {}
                 for sg in ("m", "p"):
                     qc = s2.tile([128, 32, 2], F32, tag=f"qc{sg}")
                     TS(flat(qc), flat(iy[sg]), 2.0, 1.0, ALU.mult, ALU.add)
                     TS(flat(qc), flat(qc), 1.0 / 64.0, -1.0, ALU.mult, ALU.add)
                     rr = s2.tile([128, 32, 2], F32, tag=f"rel{sg}")
                     TT(flat(rr), flat(ct), flat(qc), ALU.subtract)
                     TS(flat(rr), flat(rr), 64.0, None, ALU.mult)
                     rel[sg] = rr
                 rc = s2.tile([128, 32, 2], F32)
                 TS(flat(rc), flat(cl), 64.0, None, ALU.mult)
                 # floor-based bilinear corners y0f/y1f per axis + weights
                 tb = s2.tile([128, 32, 2], F32, tag="tbf")
                 TS(flat(tb), flat(pyc), MAGIC, MAGIC, ALU.add, ALU.subtract)
                 gtb = s2.tile([128, 32, 2], F32, tag="gtb")
                 TT(flat(gtb), flat(tb), flat(pyc), ALU.is_gt)
                 y0f = s2.tile([128, 32, 2], F32, tag="y0f")
                 TT(flat(y0f), flat(tb), flat(gtb), ALU.subtract)
                 y1f = s2.tile([128, 32, 2], F32, tag="y1f")
                 TS(flat(y1f), flat(y0f), 1.0, 63.0, ALU.add, ALU.min)
                 wyx = s2.tile([128, 32, 2], F32, tag="wyx")
                 TT(flat(wyx), flat(pyc), flat(y0f), ALU.subtract)
                 nwyx = s2.tile([128, 32, 2], F32, tag="nwyx")
                 TS(flat(nwyx), flat(wyx), -1.0, 1.0, ALU.mult, ALU.add)
                 CORN = [("m", "m"), ("m", "p"), ("p", "m"), ("p", "p")]
                 wcor = []
                 for k, (sy, sx) in enumerate(CORN):
                     ay = wyx if sy == "p" else nwyx
                     ax_ = wyx if sx == "p" else nwyx
                     wk = s2.tile([128, 32], F32, tag=f"wc{k}")
                     TT(wk[:, :], ay[:, :, 0], ax_[:, :, 1], ALU.mult)
                     wcor.append(wk)
                 # extras per ensemble: [rel_y(sy), rel_x(sx), rc_y, rc_x]
                 extras = []
                 for k, (sy, sx) in enumerate(CORN):
                     ex = s2.tile([128, 32, 4], F32, tag=f"ex{k}")
                     CP(ex[:, :, 0], rel[sy][:, :, 0])
                     CP(ex[:, :, 1], rel[sx][:, :, 1])
                     CP(ex[:, :, 2], rc[:, :, 0])
                     CP(ex[:, :, 3], rc[:, :, 1])
                     extras.append(ex)
                 # gather index tables: idx = iy*65 + ix ; f32 -> i16 ; wrapped layout
                 idxf = s2.tile([128, 9, 32], F32)
                 for k, (sy, sx) in enumerate(CORN):
                     STT(idxf[:, k, :], iy[sy][:, :, 0], 65.0, iy[sx][:, :, 1],
                         ALU.mult, ALU.add)
                 STT(idxf[:, 4, :], iy["c"][:, :, 0], 65.0, iy["c"][:, :, 1],
                     ALU.mult, ALU.add)
                 for k, (sy, sx) in enumerate(CORN):
                     by = y1f if sy == "p" else y0f
                     bx = y1f if sx == "p" else y0f
                     STT(idxf[:, 5 + k, :], by[:, :, 0], 64.0, bx[:, :, 1],
                         ALU.mult, ALU.add)
                 wrapped = s2.tile([128, 9, 256], I16)
                 idf = idxf[:, :, :].rearrange("p s b -> p (s b)")
                 t1 = []
                 for blk, cnt in ((0, 128), (1, 128), (2, 32)):
                     tt_ = s2.tile([cnt if cnt == 32 else 128, 128], F32, tag=f"t1{blk}")
                     pT = s2ps.tile([128, 128], F32, tag="pT1")
                     nc.tensor.transpose(pT[0:cnt, :], idf[:, 128 * blk:128 * blk + cnt],
                                         ident[:, :])
                     nc.vector.tensor_copy(tt_[0:cnt, :], pT[0:cnt, :])
                     t1.append((tt_, cnt))
                 for gi in range(8):
                     for blk, (tt_, cnt) in enumerate(t1):
                         pg = s2ps.tile([128, 128], F32, tag="pT1")
                         nc.tensor.transpose(pg[0:16, 0:cnt],
                                             tt_[0:cnt, 16 * gi:16 * gi + 16],
                                             ident[0:cnt, 0:cnt])
                         klo = 4 * blk
                         if cnt == 128:
                             nc.vector.tensor_copy(
                                 wrapped[0:16, klo:klo + 4, gi:256:8]
                                 .rearrange("p s b -> p (s b)"),
                                 pg[0:16, 0:128])
                         else:
                             nc.vector.tensor_copy(wrapped[0:16, 8, gi:256:8],
                                                   pg[0:16, 0:32])
                 for gr in range(1, 8):
                     nc.sync.dma_start(
                         wrapped[16 * gr:16 * gr + 16, :, :].rearrange("p s b -> p (s b)"),
                         wrapped[0:16, :, :].rearrange("p s b -> p (s b)"))

            # ================= STAGE 3: chunks =================
            out_sb = s2.tile([128, 32, 4], F32)
            nc.vector.memset(out_sb[:, :, :], 0.0)

            with tc.tile_pool(name="gath", bufs=2) as gp, \
                 tc.tile_pool(name="work", bufs=1) as wk, \
                 tc.tile_pool(name="mmps", bufs=2, space="PSUM") as mmps, \
                 tc.tile_pool(name="phps", bufs=1, space="PSUM") as phps, \
                 tc.tile_pool(name="lps", bufs=1, space="PSUM") as lps, \
                 tc.tile_pool(name="smps", bufs=1, space="PSUM") as smps, \
                 tc.tile_pool(name="tps", bufs=2, space="PSUM") as tps:

                def issue_gathers(c):
                    g = [None] * 5
                    for k in (4, 0, 1, 2, 3):  # center first: consumed first
                        gt = gp.tile([128, 4, ROW], F32, tag=f"g{k}")
                        nc.gpsimd.dma_gather(
                            gt[:, :, :], d_map.ap(),
                            wrapped[:, k, 32 * c:32 * (c + 1)], CH, CH, ROW)
                        g[k] = gt
                    g2 = []
                    for k in range(4):
                        gt = gp.tile([128, 4, ROWB], F32, tag=f"gb{k}")
                        nc.gpsimd.dma_gather(
                            gt[:, :, :], d_map2.ap(),
                            wrapped[:, 5 + k, 32 * c:32 * (c + 1)], CH, CH, ROWB)
                        g2.append(gt)
                    return g, g2

                NCH_RUN = NCH_DBG if STAGES >= 3 else 0
                pend = issue_gathers(0) if NCH_RUN else None
                for c in range(NCH_RUN):
                    g, g2 = pend
                    if c + 1 < NCH_RUN:
                        pend = issue_gathers(c + 1)  # prefetch next chunk

                    # ---- bilinear (query-major): independent of attention,
                    # emitted early to fill the softmax-tail bubble ----
                    bil = wk.tile([128, 4, 3], F32R, tag="bil")
                    for k in range(4):
                        for j in range(4):
                            if k == 0:
                                nc.vector.tensor_scalar(
                                    bil[:, j, :], g2[0][:, j, 0:3],
                                    wcor[0][:, 4 * c + j:4 * c + j + 1], None, ALU.mult)
                            else:
                                STT(bil[:, j, :], g2[k][:, j, 0:3],
                                    wcor[k][:, 4 * c + j:4 * c + j + 1], bil[:, j, :],
                                    ALU.mult, ALU.add)
                    bfm = wk.tile([3, 512], F32, tag="bfm")
                    ptb = tps.tile([128, 512], F32, tag="tp")
                    for j in range(4):
                        nc.tensor.transpose(ptb[0:3, 128 * j:128 * (j + 1)].bitcast(F32R),
                                            bil[:, j, :], identr[:, :])
                    CPA(bfm[:, :], ptb[0:3, :])

                    # ---- rc rows -> FM for phase matmul ----
                    rcfm = wk.tile([2, 512], F32R, tag="rcfm")
                    ptr = tps.tile([128, 512], F32, tag="tp")
                    for j in range(4):
                        nc.tensor.transpose(ptr[0:2, 128 * j:128 * (j + 1)],
                                            extras[0][:, 4 * c + j, 2:4], ident[:, :])
                    CPA(rcfm[:, :], ptr[0:2, :])
                    phase = phps.tile([128, 512], F32, tag="ph")
                    for j in range(4):
                        nc.tensor.matmul(phase[:, 128 * j:128 * (j + 1)],
                                         rcfm[:, 128 * j:128 * (j + 1)],
                                         pwT[:, :], start=True, stop=True)

                    # ---- center path: query ----
                    fwc = wk.tile([128, 512], F32, tag="fw")
                    TT(fwc[:, :].rearrange("p (b r) -> p b r", b=4),
                       g[4][:, :, 128:384:2], g[4][:, :, 129:384:2], ALU.add)
                    qv = wk.tile([128, 4, 256], F32, tag="kvq")

                    def enc_mul(fw, gt, dst, sfx):
                        # fw (128,512) = f (4 blocks x 128); gt gathered; dst (128,4,256) = coef*enc
                        a = wk.tile([128, 512], F32, tag="ra")
                        nc.gpsimd.tensor_scalar(a[:, :], fw[:, :], 0.5, MAGIC,
                                                ALU.mult, ALU.add)
                        k2 = wk.tile([128, 512], F32, tag="ftmp")
                        nc.gpsimd.tensor_scalar(k2[:, :], a[:, :], 2.0, -2.0 * MAGIC,
                                                ALU.mult, ALU.add)
                        fr = wk.tile([128, 512], F32, tag="rf")
                        TT(fr[:, :], fw[:, :], k2[:, :], ALU.subtract)
                        sn = wk.tile([128, 512], F32, tag="rs")
                        ACT(sn[:, :], fr[:, :], AF.Sin, scale=c_pi[:, :])
                        ab = wk.tile([128, 512], F32, tag="ra")
                        ACT(ab[:, :], fr[:, :], AF.Abs)
                        cs = wk.tile([128, 512], F32, tag="rc")
                        ACT(cs[:, :], ab[:, :], AF.Sin, scale=c_npi[:, :], bias=c_hpi[:, :])
                        cs4 = cs[:, :].rearrange("p (b r) -> p b r", b=4)
                        sn4 = sn[:, :].rearrange("p (b r) -> p b r", b=4)
                        c16 = gt[:, :, 0:128].bitcast(F16)  # (128,4,256) packed coef
                        TT(dst[:, :, 0:128], c16[:, :, 0:128], cs4, ALU.mult)
                        TT(dst[:, :, 128:256], c16[:, :, 128:256], sn4, ALU.mult)

                    enc_mul(fwc, g[4], qv, "c")
                    qfm = []
                    for blk in range(2):
                        qf = wk.tile([128, 512], F32R, tag=f"qfm{blk}")
                        ptq = tps.tile([128, 512], F32, tag="tp")
                        for j in range(4):
                            nc.tensor.transpose(ptq[:, 128 * j:128 * (j + 1)],
                                                qv[:, j, 128 * blk:128 * (blk + 1)],
                                                ident[:, :])
                        CPA(qf[:, :], ptq[:, :])
                        qfm.append(qf)

                    Lg = lps.tile([4, 512], F32, tag="lg")
                    pvs = []
                    for e, (sy, sx) in enumerate(CORN):
                        ge = g[e]
                        exfm_e = wk.tile([4, 512], F32R, tag=f"exfm{e % 2}")
                        pte = tps.tile([128, 512], F32, tag="tp")
                        for j in range(4):
                            nc.tensor.transpose(pte[0:4, 128 * j:128 * (j + 1)],
                                                extras[e][:, 4 * c + j, :], ident[:, :])
                        CPA(exfm_e[:, :], pte[0:4, :])
                        fw = wk.tile([128, 512], F32, tag=f"fw{e % 2}")
                        fw4 = fw[:, :].rearrange("p (b r) -> p b r", b=4)
                        for j in range(4):
                            tmp = wk.tile([128, 128], F32, tag=f"fj{e % 2}")
                            nc.vector.tensor_scalar(
                                tmp[:, :], ge[:, j, 129:384:2],
                                extras[e][:, 4 * c + j, 1:2], None, ALU.mult)
                            STT(fw4[:, j, :], ge[:, j, 128:384:2],
                                extras[e][:, 4 * c + j, 0:1], tmp[:, :],
                                ALU.mult, ALU.add)
                        TT(fw[:, :], fw[:, :], phase[:, :], ALU.add)
                        kv = wk.tile([128, 4, 256], F32, tag="kve")
                        enc_mul(fw, ge, kv, str(e % 2))
                        kvfm = []
                        for blk in range(2):
                            kf = wk.tile([128, 512], F32R, tag=f"kvfm{e % 2}_{blk}")
                            ptk = tps.tile([128, 512], F32, tag="tp")
                            for j in range(4):
                                nc.tensor.transpose(ptk[:, 128 * j:128 * (j + 1)],
                                                    kv[:, j, 128 * blk:128 * (blk + 1)],
                                                    ident[:, :])
                            if blk == 0:
                                CPA(kf[:, :], ptk[:, :])
                            else:
                                CP(kf[:, :], ptk[:, :])
                            kvfm.append(kf)

                        def mlp(w1n, b1, w2n, b2, tagp, po_eng):
                            h1s = []
                            for mt in range(2):
                                ms = slice(128 * mt, 128 * (mt + 1))
                                h1 = mmps.tile([128, 512], F32, tag="mm")
                                nc.tensor.matmul(h1[:, :], wsb[f"{w1n}_0"][:, ms],
                                                 kvfm[0][:, :], start=True, stop=False)
                                nc.tensor.matmul(h1[:, :], wsb[f"{w1n}_1"][:, ms],
                                                 kvfm[1][:, :], start=False, stop=False)
                                nc.tensor.matmul(h1[:, :], wsb[f"{w1n}_2"][:, ms],
                                                 exfm_e[:, :], start=False, stop=True)
                                hs = wk.tile([128, 512], F32R, tag=f"h1s{mt}")
                                ACT(hs[:, :], h1[:, :], AF.Relu,
                                    bias=b1[:, mt:mt + 1])
                                h1s.append(hs)
                            outs = []
                            for mt in range(2):
                                ms = slice(128 * mt, 128 * (mt + 1))
                                h2 = mmps.tile([128, 512], F32, tag="mm")
                                nc.tensor.matmul(h2[:, :], wsb[f"{w2n}_0"][:, ms],
                                                 h1s[0][:, :], start=True, stop=False)
                                nc.tensor.matmul(h2[:, :], wsb[f"{w2n}_1"][:, ms],
                                                 h1s[1][:, :], start=False, stop=True)
                                po = wk.tile([128, 512],
                                             F16 if tagp.startswith("pv") else F32,
                                             tag=f"{tagp}{mt}")
                                po_eng.scalar_tensor_tensor(
                                    po[:, :], h2[:, :], b2[:, mt:mt + 1],
                                    kvfm[mt][:, :], ALU.add, ALU.mult)
                                outs.append(po)
                            return outs

                        pk = mlp("kw1", kb1, "kw2", kb2, "pk", nc.vector)
                        pv = mlp("vw1", vb1, "vw2", vb2, f"pv{e}_", nc.vector)
                        pvs.append(pv)
                        for mt in range(2):
                            pr = wk.tile([128, 512], F32R, tag=f"pr{mt}")
                            eng = nc.gpsimd if mt == 0 else nc.vector
                            eng.tensor_tensor(pr[:, :], qfm[mt][:, :], pk[mt][:, :],
                                              ALU.mult)
                            nc.tensor.matmul(Lg[:, :], ehot[e][:, :], pr[:, :],
                                             start=(e == 0 and mt == 0),
                                             stop=(e == 3 and mt == 1))

                    # ---- softmax over 4 (partition dim of Lg) ----
                    Lc = wk.tile([4, 512], F32, tag="dsb")
                    TS(Lc[:, :], Lg[:, :], 80.0, None, ALU.min)
                    Ee = wk.tile([4, 512], F32R, tag="ee")
                    ACT(Ee[:, :], Lc[:, :], AF.Exp)
                    den = smps.tile([1, 512], F32, tag="sm")
                    nc.tensor.matmul(den[:, :], ones4[:, :], Ee[:, :], start=True, stop=True)
                    rden = wk.tile([1, 512], F32R, tag="rd")
                    with nc.allow_low_precision(reason="softmax denom recip, f32r out"):
                        nc.vector.reciprocal(rden[:, :], den[:, :])
                    xfm = []
                    for mt in range(2):
                        xf = wk.tile([128, 512], F32, tag=f"hqs{mt}")
                        xfm.append(xf)
                    for e in range(4):
                        bc = smps.tile([128, 512], F32, tag="bc")
                        nc.tensor.matmul(bc[:, :], rowsel[e][:, :], Ee[:, :],
                                         start=True, stop=True)
                        for mt in range(2):
                            if e == 0:
                                TT(xfm[mt][:, :], pvs[0][mt][:, :], bc[:, :], ALU.mult)
                            else:
                                tmp = wk.tile([128, 512], F32, tag=f"wt2{mt}")
                                TT(tmp[:, :], pvs[e][mt][:, :], bc[:, :], ALU.mult)
                                nc.gpsimd.tensor_tensor(xfm[mt][:, :], xfm[mt][:, :],
                                                        tmp[:, :], ALU.add)
                    rbc = smps.tile([128, 512], F32, tag="bc")
                    nc.tensor.matmul(rbc[:, :], ones_row[:, :], rden[:, :],
                                     start=True, stop=True)
                    xfr = []
                    for mt in range(2):
                        xr = wk.tile([128, 512], F32R, tag=f"xfr{mt}")
                        TT(xr[:, :], xfm[mt][:, :], rbc[:, :], ALU.mult)
                        xfr.append(xr)

                    # ---- final MLP ----
                    hqs = []
                    for mt in range(2):
                        hq = mmps.tile([128, 512], F32, tag="mm")
                        nc.tensor.matmul(hq[:, :], wsb["qw1_0"][:, 128 * mt:128 * (mt + 1)],
                                         xfr[0][:, :], start=True, stop=False)
                        nc.tensor.matmul(hq[:, :], wsb["qw1_1"][:, 128 * mt:128 * (mt + 1)],
                                         xfr[1][:, :], start=False, stop=True)
                        hs = wk.tile([128, 512], F32R, tag=f"hqs{mt}")
                        ACT(hs[:, :], hq[:, :], AF.Relu, bias=qb1[:, mt:mt + 1])
                        hqs.append(hs)
                    ops = smps.tile([3, 512], F32, tag="sm")
                    nc.tensor.matmul(ops[:, :], wsb["qw2_0"][:, :], hqs[0][:, :],
                                     start=True, stop=False)
                    nc.tensor.matmul(ops[:, :], wsb["qw2_1"][:, :], hqs[1][:, :],
                                     start=False, stop=True)
                    res = wk.tile([3, 512], F32, tag="res")
                    STT(res[:, :], ops[:, :], qb2[:, :], bfm[:, :], ALU.add, ALU.add)
                    # back to query-major out staging
                    pto = tps.tile([128, 512], F32, tag="tp")
                    for j in range(4):
                        nc.tensor.transpose(pto[:, 3 * j:3 * j + 3],
                                            res[:, 128 * j:128 * (j + 1)],
                                            ident[0:3, 0:3])
                    CPA(out_sb[:, 4 * c:4 * c + 4, 0:3],
                        pto[:, 0:12].rearrange("p (j c) -> p j c", j=4))

            nc.sync.dma_start(d_out.ap().rearrange("(p b) c -> p b c", p=128), out_sb[:, :, :])

    nc.compile()
    return nc


def _prep_inputs(inputs):
    """Host-side: slice/shard + weight layout prep. Returns list of 8 in_maps."""
    inp = np.asarray(inputs['inp'], np.float32)
    coord = np.asarray(inputs['coord'], np.float32)
    cell = np.asarray(inputs['cell'], np.float32)
    enc_w = np.asarray(inputs['enc_w'], np.float32)
    enc_b = np.asarray(inputs['enc_b'], np.float32)
    coef_w = np.asarray(inputs['coef_w'], np.float32)
    coef_b = np.asarray(inputs['coef_b'], np.float32)
    freq_w = np.asarray(inputs['freq_w'], np.float32)
    freq_b = np.asarray(inputs['freq_b'], np.float32)

    wenc = np.zeros((27, 64), np.float32)
    TAPS = [4, 0, 1, 2, 3, 5, 6, 7, 8]
    for i, t in enumerate(TAPS):
        dy, dx = t // 3, t % 3
        for ci in range(3):
            wenc[3 * i + ci] = enc_w[:, ci, dy, dx]
    encb = enc_b.reshape(64, 1)

    wcf = np.concatenate([coef_w, freq_w], axis=0)  # (512, 64, 3, 3)
    wpair = np.zeros((128, 3, 512), np.float32)
    wsing = np.zeros((64, 3, 512), np.float32)
    for dxi in range(3):
        wpair[0:64, dxi] = wcf[:, :, 0, dxi].T      # dy=-1
        wpair[64:128, dxi] = wcf[:, :, 1, dxi].T    # dy=0
        wsing[:, dxi] = wcf[:, :, 2, dxi].T         # dy=+1
    wpair = wpair.reshape(128, 3 * 512)
    wsing = wsing.reshape(64, 3 * 512)
    cfb = np.concatenate([coef_b, freq_b]).reshape(1, 512)

    inp_pad = np.zeros((B, 3, 66, 66), np.float32)
    inp_pad[:, :, 1:65, 1:65] = inp

    base = {
        'wenc': wenc, 'encb': encb, 'wpair': wpair, 'wsing': wsing, 'cfb': cfb,
        'kb1': np.asarray(inputs['kb1'], np.float32).reshape(2, 128).T.copy(),
        'kb2': np.asarray(inputs['kb2'], np.float32).reshape(2, 128).T.copy(),
        'vb1': np.asarray(inputs['vb1'], np.float32).reshape(2, 128).T.copy(),
        'vb2': np.asarray(inputs['vb2'], np.float32).reshape(2, 128).T.copy(),
        'qb1': np.asarray(inputs['qb1'], np.float32).reshape(2, 128).T.copy(),
        'qb2': np.asarray(inputs['qb2'], np.float32).reshape(3, 1),
        'pwT': np.asarray(inputs['phase_w'], np.float32).T.copy(),  # (2,128)
    }
    for nm in ('kw1', 'vw1'):
        w = np.asarray(inputs[nm], np.float32)
        base[f'{nm}_0'] = w[0:128].copy()
        base[f'{nm}_1'] = w[128:256].copy()
        base[f'{nm}_2'] = w[256:260].copy()
    for nm in ('kw2', 'vw2', 'qw1', 'qw2'):
        w = np.asarray(inputs[nm], np.float32)
        base[f'{nm}_0'] = w[0:128].copy()
        base[f'{nm}_1'] = w[128:256].copy()
    maps = []
    for c in range(NCORE):
        b, k = c // 4, c % 4
        m = dict(base)
        m['inp_pad'] = inp_pad[b].reshape(3, 66 * 66).copy()
        m['coordq'] = coord[b, k * QPC:(k + 1) * QPC].copy()
        m['cellq'] = cell[b, k * QPC:(k + 1) * QPC].copy()
        m['cell00'] = cell[b, 0:1, :].copy()
        maps.append(m)
    return maps


def kernel(**inputs):
    from concourse.bass_utils import run_bass_kernel_spmd
    if 'nc' not in _cache:
        _cache['nc'] = _build()
    nc = _cache['nc']
    in_maps = _prep_inputs(inputs)
    res = run_bass_kernel_spmd(nc, in_maps, core_ids=list(range(NCORE)))
    out = np.zeros((B, Q, 3), np.float32)
    for c in range(NCORE):
        b, k = c // 4, c % 4
        out[b, k * QPC:(k + 1) * QPC] = res.results[c]['outq'][:, :3]
    return out



# revision 71
# speedup vs baseline: 1.2015x; 1.0056x over previous
"""Trainium2 Bass kernel for nn_ATTNLTE (local-ensemble sparse attention super-resolution).

Sharding: 8 cores. Core c -> batch c//4, query quarter c%4 (4096 queries each).
Device pipeline per core:
  Stage 1: conv encoder (3->64) + coef/freq convs (64->512) via shift-trick
           matmuls, written to a padded DRAM map (65x65 rows, 576 f32 each:
           [coef 256 | freq 256 | inp 3 | pad]).  Pad row/col replicate edge.
  Stage 2: per-query index/scalar math in (128, 32) query-major layout
           (query q = p*32 + b  <->  partition p, free block b).
  Stage 3: per 512-query chunk: 5 dma_gathers (center + 4 corners), elementwise
           query-major, PE-transpose to feature-major for MLPs, attention,
           final MLP, bilinear border sample, output.
"""
import os
import sys
sys.path.insert(0, '/opt/trn_rl_repo')
import numpy as np

STAGES = int(os.environ.get('KSTAGES', '3'))
NCH_DBG = int(os.environ.get('KCHUNKS', '8'))
S1P = os.environ.get('KS1P', 'f32r')  # stage-1 conv precision: f32r | f32

B, H, W, Q = 2, 64, 64, 16384
HID = 256
NCORE = 8
QPC = 4096
NCHUNK = 8
CH = 512
ROW = 384   # map row: [coef: 128 f32 words = 256 packed fp16 | freq: 256 f32]
ROWB = 64
NMAP = 65 * 65
PI = float(np.pi)
MAGIC = 12582912.0
CLIP = 1.0 - 1e-6
EPS = 1e-6

_cache = {}


def _build():
    import concourse.bacc as bacc
    import concourse.mybir as mybir
    import concourse.tile as tile
    from concourse.alu_op_type import AluOpType as ALU
    from concourse.library_config import mlp as mlp_lib

    F32 = mybir.dt.float32
    F32R = mybir.dt.float32r  # ~1.5e-3 err, fine for the 2e-2 gate; 4x PE rate
    I16 = mybir.dt.int16
    I32 = mybir.dt.int32
    AF = mybir.ActivationFunctionType

    nc = bacc.Bacc("TRN2", target_bir_lowering=False, debug=False, num_devices=NCORE)

    d_inp_pad = nc.dram_tensor("inp_pad", (3, 66 * 66), F32, kind="ExternalInput")
    d_coord = nc.dram_tensor("coordq", (QPC, 2), F32, kind="ExternalInput")
    d_cell = nc.dram_tensor("cellq", (QPC, 2), F32, kind="ExternalInput")
    d_cell00 = nc.dram_tensor("cell00", (1, 2), F32, kind="ExternalInput")
    d_wenc = nc.dram_tensor("wenc", (27, 64), F32, kind="ExternalInput")
    d_encb = nc.dram_tensor("encb", (64, 1), F32, kind="ExternalInput")
    d_wpair = nc.dram_tensor("wpair", (128, 3 * 512), F32, kind="ExternalInput")
    d_wsing = nc.dram_tensor("wsing", (64, 3 * 512), F32, kind="ExternalInput")
    d_cfb = nc.dram_tensor("cfb", (1, 512), F32, kind="ExternalInput")
    d_w = {}
    for nm in ("kw1", "vw1"):
        for kt, p in ((0, 128), (1, 128), (2, 4)):
            d_w[f"{nm}_{kt}"] = nc.dram_tensor(f"{nm}_{kt}", (p, 256), F32, kind="ExternalInput")
    for nm in ("kw2", "vw2", "qw1"):
        for kt in range(2):
            d_w[f"{nm}_{kt}"] = nc.dram_tensor(f"{nm}_{kt}", (128, 256), F32, kind="ExternalInput")
    for kt in range(2):
        d_w[f"qw2_{kt}"] = nc.dram_tensor(f"qw2_{kt}", (128, 3), F32, kind="ExternalInput")
    d_kb1 = nc.dram_tensor("kb1", (128, 2), F32, kind="ExternalInput")
    d_kb2 = nc.dram_tensor("kb2", (128, 2), F32, kind="ExternalInput")
    d_vb1 = nc.dram_tensor("vb1", (128, 2), F32, kind="ExternalInput")
    d_vb2 = nc.dram_tensor("vb2", (128, 2), F32, kind="ExternalInput")
    d_qb1 = nc.dram_tensor("qb1", (128, 2), F32, kind="ExternalInput")
    d_qb2 = nc.dram_tensor("qb2", (3, 1), F32, kind="ExternalInput")
    d_pwT = nc.dram_tensor("pwT", (2, 128), F32, kind="ExternalInput")

    F16 = mybir.dt.float16
    d_map = nc.dram_tensor("mapt", (NMAP, ROW), F32, kind="Internal")
    d_map2 = nc.dram_tensor("mapb", (4096, ROWB), F32, kind="Internal")
    d_out = nc.dram_tensor("outq", (QPC, 4), F32, kind="ExternalOutput")

    map3 = d_map.ap().rearrange("(y x) r -> y x r", x=65)

    with tile.TileContext(nc) as tc:
        with tc.tile_pool(name="const", bufs=1) as cpool, \
             tc.tile_pool(name="wpool", bufs=1) as wpool, \
             tc.tile_pool(name="s2", bufs=1) as s2:

            nc.gpsimd.load_library(mlp_lib)

            def ccol(val, p=128):
                t = cpool.tile([p, 1], F32, tag=f"c_{val}_{p}")
                nc.vector.memset(t[:, :], float(val))
                return t

            c_pi = ccol(PI); c_hpi = ccol(PI / 2); c_npi = ccol(-PI)
            c_half = ccol(0.5); c_2 = ccol(2.0); c_n2M = ccol(-2.0 * MAGIC)
            c_M = ccol(MAGIC); c_one = ccol(1.0)
            ones_row_f = cpool.tile([1, 128], F32)
            nc.vector.memset(ones_row_f[:, :], 1.0)
            ones_row = cpool.tile([1, 128], F32R)
            nc.vector.tensor_copy(ones_row[:, :], ones_row_f[:, :])
            ones4f = cpool.tile([4, 1], F32)
            nc.vector.memset(ones4f[:, :], 1.0)
            ones4 = cpool.tile([4, 1], F32R)
            nc.vector.tensor_copy(ones4[:, :], ones4f[:, :])
            iot_p = cpool.tile([128, 128], I32)
            nc.gpsimd.iota(iot_p[:, :], [[0, 128]], channel_multiplier=1)
            iot_j = cpool.tile([128, 128], I32)
            nc.gpsimd.iota(iot_j[:, :], [[1, 128]], channel_multiplier=0)
            ident = cpool.tile([128, 128], F32)
            nc.vector.tensor_tensor(ident[:, :], iot_p[:, :], iot_j[:, :], ALU.is_equal)
            identr = cpool.tile([128, 128], F32R)
            nc.vector.tensor_copy(identr[:, :], ident[:, :])  # 1.5 cyc/row transposes
            ehot = []
            ehf = cpool.tile([128, 4], F32, tag="ehf")
            for e in range(4):
                nc.vector.memset(ehf[:, :], 0.0)
                nc.vector.tensor_copy(ehf[:, e:e + 1], c_one[:, :])
                t = cpool.tile([128, 4], F32R, tag=f"ehot{e}")
                nc.vector.tensor_copy(t[:, :], ehf[:, :])
                ehot.append(t)
            iot4 = cpool.tile([4, 128], I32)
            nc.gpsimd.iota(iot4[:, :], [[0, 128]], channel_multiplier=1)
            iot4f = cpool.tile([4, 128], F32)
            nc.vector.tensor_copy(iot4f[:, :], iot4[:, :])
            rowsel = []
            for e in range(4):
                t = cpool.tile([4, 128], F32R, tag=f"rsel{e}")
                nc.vector.tensor_scalar(t[:, :], iot4f[:, :], float(e), None, ALU.is_equal)
                rowsel.append(t)

            def wtile(dap, p, n, tag, rdt=None):
                if rdt is None:
                    t = wpool.tile([p, n], F32, tag=tag)
                    nc.sync.dma_start(t[:, :], dap)
                    return t
                t = wpool.tile([128, 256], F32, tag="wstg")
                nc.sync.dma_start(t[0:p, 0:n], dap)
                tr = wpool.tile([p, n], rdt, tag=tag + "_r")
                nc.vector.tensor_copy(tr[:, :], t[0:p, 0:n])
                return tr

            wsb = {}
            for nm in ("kw1", "vw1"):
                for kt, p in ((0, 128), (1, 128), (2, 4)):
                    wsb[f"{nm}_{kt}"] = wtile(d_w[f"{nm}_{kt}"].ap(), p, 256, f"{nm}_{kt}", F32R)
            for nm in ("kw2", "vw2", "qw1"):
                for kt in range(2):
                    wsb[f"{nm}_{kt}"] = wtile(d_w[f"{nm}_{kt}"].ap(), 128, 256, f"{nm}_{kt}", F32R)
            for kt in range(2):
                wsb[f"qw2_{kt}"] = wtile(d_w[f"qw2_{kt}"].ap(), 128, 3, f"qw2_{kt}", F32R)
            kb1 = wtile(d_kb1.ap(), 128, 2, "kb1"); kb2 = wtile(d_kb2.ap(), 128, 2, "kb2")
            vb1 = wtile(d_vb1.ap(), 128, 2, "vb1"); vb2 = wtile(d_vb2.ap(), 128, 2, "vb2")
            qb1 = wtile(d_qb1.ap(), 128, 2, "qb1"); qb2 = wtile(d_qb2.ap(), 3, 1, "qb2")
            pwT = wtile(d_pwT.ap(), 2, 128, "pwT", F32R)

            TT = nc.vector.tensor_tensor
            TS = nc.vector.tensor_scalar
            STT = nc.vector.scalar_tensor_tensor
            ACT = nc.scalar.activation
            CP = nc.vector.tensor_copy
            CPA = nc.scalar.copy
            I32 = mybir.dt.int32

            def recip_newton(pool, out, in_, tag, iters=3):
                p, n = in_.shape[0], int(np.prod(in_.shape[1:]))
                r = pool.tile([p, n], F32, tag=f"{tag}r")
                TS(r[:, :].bitcast(I32), in_.bitcast(I32), -1, 0x7EF311C2,
                   ALU.mult, ALU.add)
                t = pool.tile([p, n], F32, tag=f"{tag}t")
                for _ in range(iters):
                    TT(t[:, :], in_, r[:, :], ALU.mult)
                    TS(t[:, :], t[:, :], -1.0, 2.0, ALU.mult, ALU.add)
                    TT(r[:, :], r[:, :], t[:, :], ALU.mult)
                CP(out, r[:, :])

            # ================= STAGE 1: convs -> map =================
            if STAGES >= 1:
              with tc.tile_pool(name="conv", bufs=1) as cv, \
                  tc.tile_pool(name="cst", bufs=3) as cstage, \
                  tc.tile_pool(name="cps", bufs=2, space="PSUM") as cps, \
                  tc.tile_pool(name="cpt", bufs=2, space="PSUM") as cpt:
                 def cvt(dap, p, n, tag, rdt=None):
                     t = cv.tile([p, n], F32, tag=tag)
                     nc.sync.dma_start(t[:, :], dap)
                     if rdt is None:
                         return t
                     tr = cv.tile([p, n], rdt, tag=tag + "_r")
                     nc.vector.tensor_copy(tr[:, :], t[:, :])
                     return tr
                 R1 = F32R if S1P == 'f32r' else None
                 wenc = cvt(d_wenc.ap(), 27, 64, "wenc", R1)
                 encb = cvt(d_encb.ap(), 64, 1, "encb")
                 wpair = cvt(d_wpair.ap(), 128, 3 * 512, "wpair", R1)
                 wsing = cvt(d_wsing.ap(), 64, 3 * 512, "wsing", R1)
                 cfb = cvt(d_cfb.ap(), 1, 512, "cfb", R1)
                 im2 = cv.tile([27, 4096], F32)
                 inp3 = d_inp_pad.ap().rearrange("c (h w) -> c h w", h=66)
                 TAPS = [4, 0, 1, 2, 3, 5, 6, 7, 8]
                 for i, t in enumerate(TAPS):
                     dy, dx = t // 3 - 1, t % 3 - 1
                     nc.sync.dma_start(
                         im2[3 * i:3 * i + 3, :].rearrange("c (h w) -> c h w", h=64),
                         inp3[:, 1 + dy:65 + dy, 1 + dx:65 + dx])
                 if S1P == 'f32r':
                     im2r = cv.tile([27, 4096], F32R, tag="im2r")
                     CPA(im2r[:, :], im2[:, :])
                 else:
                     im2r = im2
                 fpad = cv.tile([128, 66 * 66], F32)
                 nc.vector.memset(fpad[:, :], 0.0)
                 f3a = fpad[0:64, :].rearrange("c (h w) -> c h w", h=66)
                 f3b = fpad[64:128, :].rearrange("c (h w) -> c h w", h=66)
                 for nt in range(8):
                     pf = cps.tile([64, 512], F32, tag="pf")
                     nc.tensor.matmul(pf[:, :], wenc[:, :],
                                      im2r[:, 512 * nt:512 * (nt + 1)],
                                      start=True, stop=True)
                     y0 = nt * 8
                     pf3 = pf[:, :].rearrange("c (h w) -> c h w", h=8)
                     ACT(f3a[:, y0 + 1:y0 + 9, 1:65], pf3, AF.Identity, bias=encb[:, :])
                     ACT(f3b[:, y0:y0 + 8, 1:65], pf3, AF.Identity, bias=encb[:, :])
                 fp3 = fpad[:, :].rearrange("c (h w) -> c h w", h=66)
                 imcf = []
                 for dxi, dx in enumerate((-1, 0, 1)):
                     t = cv.tile([128, 66 * 64], F32R if S1P == 'f32r' else F32,
                                 tag=f"imcf{dxi}")
                     nc.vector.tensor_copy(
                         t[:, :].rearrange("c (h w) -> c h w", h=66),
                         fp3[:, 0:66, 1 + dx:65 + dx])
                     imcf.append(t)
                 for rt in range(32):
                     y0 = 2 * rt
                     pcf = cps.tile([128, 512], F32, tag="pcf")
                     first = True
                     for dxi in range(3):
                         nc.tensor.matmul(pcf[:, :],
                                          imcf[dxi][:, 128 * rt:128 * rt + 128],
                                          wpair[:, 512 * dxi:512 * (dxi + 1)],
                                          start=first, stop=False)
                         first = False
                         nc.tensor.matmul(pcf[:, :],
                                          imcf[dxi][0:64, 128 * rt + 128:128 * rt + 256],
                                          wsing[:, 512 * dxi:512 * (dxi + 1)],
                                          start=False, stop=False)
                     nc.tensor.matmul(pcf[:, :],
                                      (ones_row if S1P == 'f32r' else ones_row_f)[:, :],
                                      cfb[:, :], start=False, stop=True)
                     ti = cpt.tile([128, 128], F32, tag="ti")
                     nc.tensor.transpose(ti[:, 0:3], im2[0:3, 128 * rt:128 * (rt + 1)],
                                         ident[0:3, 0:3])
                     st = cstage.tile([128, ROW], F32, tag="st")
                     CPA(st[:, 0:128].bitcast(F16), pcf[:, 0:256])
                     CPA(st[:, 128:384], pcf[:, 256:512])
                     nc.sync.dma_start(map3[y0:y0 + 2, 0:64, :], st[:, :])
                     st2 = cstage.tile([128, ROWB], F32, tag="st2")
                     CP(st2[:, 0:3], ti[:, 0:3])
                     nc.sync.dma_start(
                         d_map2.ap()[128 * rt:128 * (rt + 1), :], st2[:, :])
                 # pad col 64 <- col 63 ; then pad row 64 <- row 63 (covers corner)
                 nc.sync.dma_start(map3[0:64, 64:65, :], map3[0:64, 63:64, :])
                 nc.sync.dma_start(map3[64:65, :, :], map3[63:64, :, :])

            # ================= STAGE 2: per-query scalars =================
            if STAGES >= 2:
             ct = s2.tile([128, 32, 2], F32)
             nc.sync.dma_start(ct[:, :, :], d_coord.ap().rearrange("(p b) c -> p b c", p=128))
             cl = s2.tile([128, 32, 2], F32)
             nc.sync.dma_start(cl[:, :, :], d_cell.ap().rearrange("(p b) c -> p b c", p=128))
             c00 = s2.tile([1, 2], F32)
             nc.sync.dma_start(c00[:, :], d_cell00.ap())

             with tc.tile_pool(name="s2ps", bufs=2, space="PSUM") as s2ps:
                 # rx = 1/(63/(1-c00)) per axis, shift scalars, broadcast to cols
                 u = s2.tile([1, 2], F32)
                 TS(u[:, :], c00[:, :], -1.0, 1.0, ALU.mult, ALU.add)          # 1 - c00
                 iu = s2.tile([1, 2], F32)
                 recip_newton(s2, iu[:, :], u[:, :], "riu")
                 txy = s2.tile([1, 2], F32)
                 TS(txy[:, :], iu[:, :], 63.0, None, ALU.mult)                  # ~63/(1-c00)
                 rxy = s2.tile([1, 2], F32)
                 recip_newton(s2, rxy[:, :], txy[:, :], "rxy")                  # ~1/t
                 shp = s2.tile([1, 2], F32)
                 TS(shp[:, :], rxy[:, :], 1.0, EPS, ALU.mult, ALU.add)          # +r + eps
                 shm = s2.tile([1, 2], F32)
                 TS(shm[:, :], rxy[:, :], -1.0, EPS, ALU.mult, ALU.add)         # -r + eps
                 shcol = {}
                 for sg, t in (("p", shp), ("m", shm)):
                     for ax in range(2):
                         ps = s2ps.tile([128, 1], F32, tag="sb")
                         nc.tensor.matmul(ps[:, :], ones_row_f[:, :], t[:, ax:ax + 1],
                                          start=True, stop=True)
                         col = s2.tile([128, 1], F32, tag=f"shc{sg}{ax}")
                         CP(col[:, :], ps[:, :])
                         shcol[(sg, ax)] = col

                 def flat(t):
                     return t[:, :, :].rearrange("p b c -> p (b c)")

                 # center py/px and iy/ix; shifted iy/ix; rel; idx; bilinear weights
                 iy = {}
                 pyc = s2.tile([128, 32, 2], F32)   # clip(py, 0, 63) both axes
                 u1 = s2.tile([128, 32, 2], F32, tag="u1")
                 TS(flat(u1), flat(ct), 1.0, 32.0, ALU.add, ALU.mult)
                 t2 = s2.tile([128, 32, 2], F32, tag="t2")
                 TS(flat(t2), flat(u1), -0.5, MAGIC, ALU.add, ALU.add)
                 iyc = s2.tile([128, 32, 2], F32, tag="iyc")
                 TS(flat(iyc), flat(t2), MAGIC, 0.0, ALU.subtract, ALU.max)
                 TS(flat(iyc), flat(iyc), 63.0, None, ALU.min)
                 iy["c"] = iyc
                 pyr = s2.tile([128, 32, 2], F32, tag="pyr")
                 TS(flat(pyr), flat(u1), -0.5, None, ALU.add)
                 TS(flat(pyc), flat(pyr), 0.0, 63.0, ALU.max, ALU.min)
                 for sg in ("m", "p"):
                     cc = s2.tile([128, 32, 2], F32, tag=f"cc{sg}")
                     for ax in range(2):
                         STT(cc[:, :, ax], ct[:, :, ax], shcol[(sg, ax)][:, :],
                             None, ALU.add, ALU.bypass) if False else None
                         # c' = clip(c + s, -CLIP, CLIP)
                         nc.vector.tensor_scalar(
                             cc[:, :, ax], ct[:, :, ax], shcol[(sg, ax)][:, :], -CLIP,
                             ALU.add, ALU.max)
                     TS(flat(cc), flat(cc), CLIP, None, ALU.min)
                     uu = s2.tile([128, 32, 2], F32, tag=f"uu{sg}")
                     TS(flat(uu), flat(cc), 1.0, 32.0, ALU.add, ALU.mult)
                     tt2 = s2.tile([128, 32, 2], F32, tag=f"tt2{sg}")
                     TS(flat(tt2), flat(uu), -0.5, MAGIC, ALU.add, ALU.add)
                     ii = s2.tile([128, 32, 2], F32, tag=f"ii{sg}")
                     TS(flat(ii), flat(tt2), MAGIC, 0.0, ALU.subtract, ALU.max)
                     TS(flat(ii), flat(ii), 63.0, None, ALU.min)
                     iy[sg] = ii
                 # rel per sign/axis: rel = (c - qc)*64 ; qc = (2*iy+1)/64 - 1
                 rel = {}
                 for sg in ("m", "p"):
                     qc = s2.tile([128, 32, 2], F32, tag=f"qc{sg}")
                     TS(flat(qc), flat(iy[sg]), 2.0, 1.0, ALU.mult, ALU.add)
                     TS(flat(qc), flat(qc), 1.0 / 64.0, -1.0, ALU.mult, ALU.add)
                     rr = s2.tile([128, 32, 2], F32, tag=f"rel{sg}")
                     TT(flat(rr), flat(ct), flat(qc), ALU.subtract)
                     TS(flat(rr), flat(rr), 64.0, None, ALU.mult)
                     rel[sg] = rr
                 rc = s2.tile([128, 32, 2], F32)
                 TS(flat(rc), flat(cl), 64.0, None, ALU.mult)
                 # floor-based bilinear corners y0f/y1f per axis + weights
                 tb = s2.tile([128, 32, 2], F32, tag="tbf")
                 TS(flat(tb), flat(pyc), MAGIC, MAGIC, ALU.add, ALU.subtract)
                 gtb = s2.tile([128, 32, 2], F32, tag="gtb")
                 TT(flat(gtb), flat(tb), flat(pyc), ALU.is_gt)
                 y0f = s2.tile([128, 32, 2], F32, tag="y0f")
                 TT(flat(y0f), flat(tb), flat(gtb), ALU.subtract)
                 y1f = s2.tile([128, 32, 2], F32, tag="y1f")
                 TS(flat(y1f), flat(y0f), 1.0, 63.0, ALU.add, ALU.min)
                 wyx = s2.tile([128, 32, 2], F32, tag="wyx")
                 TT(flat(wyx), flat(pyc), flat(y0f), ALU.subtract)
                 nwyx = s2.tile([128, 32, 2], F32, tag="nwyx")
                 TS(flat(nwyx), flat(wyx), -1.0, 1.0, ALU.mult, ALU.add)
                 CORN = [("m", "m"), ("m", "p"), ("p", "m"), ("p", "p")]
                 wcor = []
                 for k, (sy, sx) in enumerate(CORN):
                     ay = wyx if sy == "p" else nwyx
                     ax_ = wyx if sx == "p" else nwyx
                     wk = s2.tile([128, 32], F32, tag=f"wc{k}")
                     TT(wk[:, :], ay[:, :, 0], ax_[:, :, 1], ALU.mult)
                     wcor.append(wk)
                 # extras per ensemble: [rel_y(sy), rel_x(sx), rc_y, rc_x]
                 extras = []
                 for k, (sy, sx) in enumerate(CORN):
                     ex = s2.tile([128, 32, 4], F32, tag=f"ex{k}")
                     CP(ex[:, :, 0], rel[sy][:, :, 0])
                     CP(ex[:, :, 1], rel[sx][:, :, 1])
                     CP(ex[:, :, 2], rc[:, :, 0])
                     CP(ex[:, :, 3], rc[:, :, 1])
                     extras.append(ex)
                 # gather index tables: idx = iy*65 + ix ; f32 -> i16 ; wrapped layout
                 idxf = s2.tile([128, 9, 32], F32)
                 for k, (sy, sx) in enumerate(CORN):
                     STT(idxf[:, k, :], iy[sy][:, :, 0], 65.0, iy[sx][:, :, 1],
                         ALU.mult, ALU.add)
                 STT(idxf[:, 4, :], iy["c"][:, :, 0], 65.0, iy["c"][:, :, 1],
                     ALU.mult, ALU.add)
                 for k, (sy, sx) in enumerate(CORN):
                     by = y1f if sy == "p" else y0f
                     bx = y1f if sx == "p" else y0f
                     STT(idxf[:, 5 + k, :], by[:, :, 0], 64.0, bx[:, :, 1],
                         ALU.mult, ALU.add)
                 wrapped = s2.tile([128, 9, 256], I16)
                 idf = idxf[:, :, :].rearrange("p s b -> p (s b)")
                 t1 = []
                 for blk, cnt in ((0, 128), (1, 128), (2, 32)):
                     tt_ = s2.tile([cnt if cnt == 32 else 128, 128], F32, tag=f"t1{blk}")
                     pT = s2ps.tile([128, 128], F32, tag="pT1")
                     nc.tensor.transpose(pT[0:cnt, :], idf[:, 128 * blk:128 * blk + cnt],
                                         ident[:, :])
                     nc.vector.tensor_copy(tt_[0:cnt, :], pT[0:cnt, :])
                     t1.append((tt_, cnt))
                 for gi in range(8):
                     for blk, (tt_, cnt) in enumerate(t1):
                         pg = s2ps.tile([128, 128], F32, tag="pT1")
                         nc.tensor.transpose(pg[0:16, 0:cnt],
                                             tt_[0:cnt, 16 * gi:16 * gi + 16],
                                             ident[0:cnt, 0:cnt])
                         klo = 4 * blk
                         if cnt == 128:
                             nc.vector.tensor_copy(
                                 wrapped[0:16, klo:klo + 4, gi:256:8]
                                 .rearrange("p s b -> p (s b)"),
                                 pg[0:16, 0:128])
                         else:
                             nc.vector.tensor_copy(wrapped[0:16, 8, gi:256:8],
                                                   pg[0:16, 0:32])
                 for gr in range(1, 8):
                     nc.sync.dma_start(
                         wrapped[16 * gr:16 * gr + 16, :, :].rearrange("p s b -> p (s b)"),
                         wrapped[0:16, :, :].rearrange("p s b -> p (s b)"))

            # ================= STAGE 3: chunks =================
            out_sb = s2.tile([128, 32, 4], F32)
            nc.vector.memset(out_sb[:, :, :], 0.0)

            with tc.tile_pool(name="gath", bufs=2) as gp, \
                 tc.tile_pool(name="work", bufs=1) as wk, \
                 tc.tile_pool(name="mmps", bufs=2, space="PSUM") as mmps, \
                 tc.tile_pool(name="phps", bufs=1, space="PSUM") as phps, \
                 tc.tile_pool(name="lps", bufs=1, space="PSUM") as lps, \
                 tc.tile_pool(name="smps", bufs=1, space="PSUM") as smps, \
                 tc.tile_pool(name="tps", bufs=2, space="PSUM") as tps:

                def issue_gathers(c):
                    g = [None] * 5
                    for k in (4, 0, 1, 2, 3):  # center first: consumed first
                        gt = gp.tile([128, 4, ROW], F32, tag=f"g{k}")
                        nc.gpsimd.dma_gather(
                            gt[:, :, :], d_map.ap(),
                            wrapped[:, k, 32 * c:32 * (c + 1)], CH, CH, ROW)
                        g[k] = gt
                    g2 = []
                    for k in range(4):
                        gt = gp.tile([128, 4, ROWB], F32, tag=f"gb{k}")
                        nc.gpsimd.dma_gather(
                            gt[:, :, :], d_map2.ap(),
                            wrapped[:, 5 + k, 32 * c:32 * (c + 1)], CH, CH, ROWB)
                        g2.append(gt)
                    return g, g2

                NCH_RUN = NCH_DBG if STAGES >= 3 else 0
                pend = issue_gathers(0) if NCH_RUN else None
                for c in range(NCH_RUN):
                    g, g2 = pend
                    if c + 1 < NCH_RUN:
                        pend = issue_gathers(c + 1)  # prefetch next chunk

                    # ---- bilinear (query-major): independent of attention,
                    # emitted early to fill the softmax-tail bubble ----
                    bil = wk.tile([128, 4, 3], F32R, tag="bil")
                    for k in range(4):
                        for j in range(4):
                            if k == 0:
                                nc.vector.tensor_scalar(
                                    bil[:, j, :], g2[0][:, j, 0:3],
                                    wcor[0][:, 4 * c + j:4 * c + j + 1], None, ALU.mult)
                            else:
                                STT(bil[:, j, :], g2[k][:, j, 0:3],
                                    wcor[k][:, 4 * c + j:4 * c + j + 1], bil[:, j, :],
                                    ALU.mult, ALU.add)
                    bfm = wk.tile([3, 512], F32, tag="bfm")
                    ptb = tps.tile([128, 512], F32, tag="tp")
                    for j in range(4):
                        nc.tensor.transpose(ptb[0:3, 128 * j:128 * (j + 1)].bitcast(F32R),
                                            bil[:, j, :], identr[:, :])
                    CPA(bfm[:, :], ptb[0:3, :])

                    # ---- rc rows -> FM for phase matmul ----
                    rcfm = wk.tile([2, 512], F32R, tag="rcfm")
                    ptr = tps.tile([128, 512], F32, tag="tp")
                    for j in range(4):
                        nc.tensor.transpose(ptr[0:2, 128 * j:128 * (j + 1)],
                                            extras[0][:, 4 * c + j, 2:4], ident[:, :])
                    CPA(rcfm[:, :], ptr[0:2, :])
                    phase = phps.tile([128, 512], F32, tag="ph")
                    for j in range(4):
                        nc.tensor.matmul(phase[:, 128 * j:128 * (j + 1)],
                                         rcfm[:, 128 * j:128 * (j + 1)],
                                         pwT[:, :], start=True, stop=True)

                    # ---- center path: query ----
                    fwc = wk.tile([128, 512], F32, tag="fw")
                    TT(fwc[:, :].rearrange("p (b r) -> p b r", b=4),
                       g[4][:, :, 128:384:2], g[4][:, :, 129:384:2], ALU.add)
                    qv = wk.tile([128, 4, 256], F32, tag="kvq")

                    def enc_mul(fw, gt, dst, sfx):
                        # fw (128,512) = f (4 blocks x 128); gt gathered; dst (128,4,256) = coef*enc
                        a = wk.tile([128, 512], F32, tag="ra")
                        nc.gpsimd.tensor_scalar(a[:, :], fw[:, :], 0.5, MAGIC,
                                                ALU.mult, ALU.add)
                        k2 = wk.tile([128, 512], F32, tag="ftmp")
                        nc.gpsimd.tensor_scalar(k2[:, :], a[:, :], 2.0, -2.0 * MAGIC,
                                                ALU.mult, ALU.add)
                        fr = wk.tile([128, 512], F32, tag="rf")
                        TT(fr[:, :], fw[:, :], k2[:, :], ALU.subtract)
                        sn = wk.tile([128, 512], F32, tag="rs")
                        ACT(sn[:, :], fr[:, :], AF.Sin, scale=c_pi[:, :])
                        ab = wk.tile([128, 512], F32, tag="ra")
                        ACT(ab[:, :], fr[:, :], AF.Abs)
                        cs = wk.tile([128, 512], F32, tag="rc")
                        ACT(cs[:, :], ab[:, :], AF.Sin, scale=c_npi[:, :], bias=c_hpi[:, :])
                        cs4 = cs[:, :].rearrange("p (b r) -> p b r", b=4)
                        sn4 = sn[:, :].rearrange("p (b r) -> p b r", b=4)
                        c16 = gt[:, :, 0:128].bitcast(F16)  # (128,4,256) packed coef
                        TT(dst[:, :, 0:128], c16[:, :, 0:128], cs4, ALU.mult)
                        TT(dst[:, :, 128:256], c16[:, :, 128:256], sn4, ALU.mult)

                    enc_mul(fwc, g[4], qv, "c")
                    qfm = []
                    for blk in range(2):
                        qf = wk.tile([128, 512], F32R, tag=f"qfm{blk}")
                        ptq = tps.tile([128, 512], F32, tag="tp")
                        for j in range(4):
                            nc.tensor.transpose(ptq[:, 128 * j:128 * (j + 1)],
                                                qv[:, j, 128 * blk:128 * (blk + 1)],
                                                ident[:, :])
                        CPA(qf[:, :], ptq[:, :])
                        qfm.append(qf)

                    Lg = lps.tile([4, 512], F32, tag="lg")
                    pvs = []
                    for e, (sy, sx) in enumerate(CORN):
                        ge = g[e]
                        exfm_e = wk.tile([4, 512], F32R, tag=f"exfm{e % 2}")
                        pte = tps.tile([128, 512], F32, tag="tp")
                        for j in range(4):
                            nc.tensor.transpose(pte[0:4, 128 * j:128 * (j + 1)],
                                                extras[e][:, 4 * c + j, :], ident[:, :])
                        CPA(exfm_e[:, :], pte[0:4, :])
                        fw = wk.tile([128, 512], F32, tag=f"fw{e % 2}")
                        fw4 = fw[:, :].rearrange("p (b r) -> p b r", b=4)
                        for j in range(4):
                            tmp = wk.tile([128, 128], F32, tag=f"fj{e % 2}")
                            nc.vector.tensor_scalar(
                                tmp[:, :], ge[:, j, 129:384:2],
                                extras[e][:, 4 * c + j, 1:2], None, ALU.mult)
                            STT(fw4[:, j, :], ge[:, j, 128:384:2],
                                extras[e][:, 4 * c + j, 0:1], tmp[:, :],
                                ALU.mult, ALU.add)
                        TT(fw[:, :], fw[:, :], phase[:, :], ALU.add)
                        kv = wk.tile([128, 4, 256], F32, tag="kve")
                        enc_mul(fw, ge, kv, str(e % 2))
                        kvfm = []
                        for blk in range(2):
                            kf = wk.tile([128, 512], F32R, tag=f"kvfm{e % 2}_{blk}")
                            ptk = tps.tile([128, 512], F32, tag="tp")
                            for j in range(4):
                                nc.tensor.transpose(ptk[:, 128 * j:128 * (j + 1)],
                                                    kv[:, j, 128 * blk:128 * (blk + 1)],
                                                    ident[:, :])
                            if blk == 0:
                                CPA(kf[:, :], ptk[:, :])
                            else:
                                CP(kf[:, :], ptk[:, :])
                            kvfm.append(kf)

                        def mlp(w1n, b1, w2n, b2, tagp, po_eng):
                            h1s = []
                            for mt in range(2):
                                ms = slice(128 * mt, 128 * (mt + 1))
                                h1 = mmps.tile([128, 512], F32, tag="mm")
                                nc.tensor.matmul(h1[:, :], wsb[f"{w1n}_0"][:, ms],
                                                 kvfm[0][:, :], start=True, stop=False)
                                nc.tensor.matmul(h1[:, :], wsb[f"{w1n}_1"][:, ms],
                                                 kvfm[1][:, :], start=False, stop=False)
                                nc.tensor.matmul(h1[:, :], wsb[f"{w1n}_2"][:, ms],
                                                 exfm_e[:, :], start=False, stop=True)
                                hs = wk.tile([128, 512], F32R, tag=f"h1s{mt}")
                                ACT(hs[:, :], h1[:, :], AF.Relu,
                                    bias=b1[:, mt:mt + 1])
                                h1s.append(hs)
                            outs = []
                            for mt in range(2):
                                ms = slice(128 * mt, 128 * (mt + 1))
                                h2 = mmps.tile([128, 512], F32, tag="mm")
                                nc.tensor.matmul(h2[:, :], wsb[f"{w2n}_0"][:, ms],
                                                 h1s[0][:, :], start=True, stop=False)
                                nc.tensor.matmul(h2[:, :], wsb[f"{w2n}_1"][:, ms],
                                                 h1s[1][:, :], start=False, stop=True)
                                po = wk.tile([128, 512],
                                             F16 if tagp.startswith("pv") else F32,
                                             tag=f"{tagp}{mt}")
                                po_eng.scalar_tensor_tensor(
                                    po[:, :], h2[:, :], b2[:, mt:mt + 1],
                                    kvfm[mt][:, :], ALU.add, ALU.mult)
                                outs.append(po)
                            return outs

                        pk = mlp("kw1", kb1, "kw2", kb2, "pk", nc.vector)
                        pv = mlp("vw1", vb1, "vw2", vb2, f"pv{e}_", nc.vector)
                        pvs.append(pv)
                        for mt in range(2):
                            pr = wk.tile([128, 512], F32R, tag=f"pr{mt}")
                            eng = nc.gpsimd if mt == 0 else nc.vector
                            eng.tensor_tensor(pr[:, :], qfm[mt][:, :], pk[mt][:, :],
                                              ALU.mult)
                            nc.tensor.matmul(Lg[:, :], ehot[e][:, :], pr[:, :],
                                             start=(e == 0 and mt == 0),
                                             stop=(e == 3 and mt == 1))

                    # ---- softmax over 4 (partition dim of Lg) ----
                    Lc = wk.tile([4, 512], F32, tag="dsb")
                    TS(Lc[:, :], Lg[:, :], 80.0, None, ALU.min)
                    Ee = wk.tile([4, 512], F32R, tag="ee")
                    ACT(Ee[:, :], Lc[:, :], AF.Exp)
                    den = smps.tile([1, 512], F32, tag="sm")
                    nc.tensor.matmul(den[:, :], ones4[:, :], Ee[:, :], start=True, stop=True)
                    rden = wk.tile([1, 512], F32R, tag="rd")
                    with nc.allow_low_precision(reason="softmax denom recip, f32r out"):
                        nc.vector.reciprocal(rden[:, :], den[:, :])
                    xfm = []
                    for mt in range(2):
                        xf = wk.tile([128, 512], F32, tag=f"hqs{mt}")
                        xfm.append(xf)
                    for e in range(4):
                        bc = smps.tile([128, 512], F32, tag="bc")
                        nc.tensor.matmul(bc[:, :], rowsel[e][:, :], Ee[:, :],
                                         start=True, stop=True)
                        for mt in range(2):
                            if e == 0:
                                TT(xfm[mt][:, :], pvs[0][mt][:, :], bc[:, :], ALU.mult)
                            else:
                                tmp = wk.tile([128, 512], F32, tag=f"wt2{mt}")
                                TT(tmp[:, :], pvs[e][mt][:, :], bc[:, :], ALU.mult)
                                nc.gpsimd.tensor_tensor(xfm[mt][:, :], xfm[mt][:, :],
                                                        tmp[:, :], ALU.add)
                    rbc = smps.tile([128, 512], F32, tag="bc")
                    nc.tensor.matmul(rbc[:, :], ones_row[:, :], rden[:, :],
                                     start=True, stop=True)
                    xfr = []
                    for mt in range(2):
                        xr = wk.tile([128, 512], F32R, tag=f"xfr{mt}")
                        TT(xr[:, :], xfm[mt][:, :], rbc[:, :], ALU.mult)
                        xfr.append(xr)

                    # ---- final MLP ----
                    hqs = []
                    for mt in range(2):
                        hq = mmps.tile([128, 512], F32, tag="mm")
                        nc.tensor.matmul(hq[:, :], wsb["qw1_0"][:, 128 * mt:128 * (mt + 1)],
                                         xfr[0][:, :], start=True, stop=False)
                        nc.tensor.matmul(hq[:, :], wsb["qw1_1"][:, 128 * mt:128 * (mt + 1)],
                                         xfr[1][:, :], start=False, stop=True)
                        hs = wk.tile([128, 512], F32R, tag=f"hqs{mt}")
                        ACT(hs[:, :], hq[:, :], AF.Relu, bias=qb1[:, mt:mt + 1])
                        hqs.append(hs)
                    ops = smps.tile([3, 512], F32, tag="sm")
                    nc.tensor.matmul(ops[:, :], wsb["qw2_0"][:, :], hqs[0][:, :],
                                     start=True, stop=False)
                    nc.tensor.matmul(ops[:, :], wsb["qw2_1"][:, :], hqs[1][:, :],
                                     start=False, stop=True)
                    res = wk.tile([3, 512], F32, tag="res")
                    STT(res[:, :], ops[:, :], qb2[:, :], bfm[:, :], ALU.add, ALU.add)
                    # back to query-major out staging
                    pto = tps.tile([128, 512], F32, tag="tp")
                    for j in range(4):
                        nc.tensor.transpose(pto[:, 3 * j:3 * j + 3],
                                            res[:, 128 * j:128 * (j + 1)],
                                            ident[0:3, 0:3])
                    CPA(out_sb[:, 4 * c:4 * c + 4, 0:3],
                        pto[:, 0:12].rearrange("p (j c) -> p j c", j=4))

            nc.sync.dma_start(d_out.ap().rearrange("(p b) c -> p b c", p=128), out_sb[:, :, :])

    nc.compile()
    return nc


def _prep_inputs(inputs):
    """Host-side: slice/shard + weight layout prep. Returns list of 8 in_maps."""
    inp = np.asarray(inputs['inp'], np.float32)
    coord = np.asarray(inputs['coord'], np.float32)
    cell = np.asarray(inputs['cell'], np.float32)
    enc_w = np.asarray(inputs['enc_w'], np.float32)
    enc_b = np.asarray(inputs['enc_b'], np.float32)
    coef_w = np.asarray(inputs['coef_w'], np.float32)
    coef_b = np.asarray(inputs['coef_b'], np.float32)
    freq_w = np.asarray(inputs['freq_w'], np.float32)
    freq_b = np.asarray(inputs['freq_b'], np.float32)

    wenc = np.zeros((27, 64), np.float32)
    TAPS = [4, 0, 1, 2, 3, 5, 6, 7, 8]
    for i, t in enumerate(TAPS):
        dy, dx = t // 3, t % 3
        for ci in range(3):
            wenc[3 * i + ci] = enc_w[:, ci, dy, dx]
    encb = enc_b.reshape(64, 1)

    wcf = np.concatenate([coef_w, freq_w], axis=0)  # (512, 64, 3, 3)
    wpair = np.zeros((128, 3, 512), np.float32)
    wsing = np.zeros((64, 3, 512), np.float32)
    for dxi in range(3):
        wpair[0:64, dxi] = wcf[:, :, 0, dxi].T      # dy=-1
        wpair[64:128, dxi] = wcf[:, :, 1, dxi].T    # dy=0
        wsing[:, dxi] = wcf[:, :, 2, dxi].T         # dy=+1
    wpair = wpair.reshape(128, 3 * 512)
    wsing = wsing.reshape(64, 3 * 512)
    cfb = np.concatenate([coef_b, freq_b]).reshape(1, 512)

    inp_pad = np.zeros((B, 3, 66, 66), np.float32)
    inp_pad[:, :, 1:65, 1:65] = inp

    base = {
        'wenc': wenc, 'encb': encb, 'wpair': wpair, 'wsing': wsing, 'cfb': cfb,
        'kb1': np.asarray(inputs['kb1'], np.float32).reshape(2, 128).T.copy(),
        'kb2': np.asarray(inputs['kb2'], np.float32).reshape(2, 128).T.copy(),
        'vb1': np.asarray(inputs['vb1'], np.float32).reshape(2, 128).T.copy(),
        'vb2': np.asarray(inputs['vb2'], np.float32).reshape(2, 128).T.copy(),
        'qb1': np.asarray(inputs['qb1'], np.float32).reshape(2, 128).T.copy(),
        'qb2': np.asarray(inputs['qb2'], np.float32).reshape(3, 1),
        'pwT': np.asarray(inputs['phase_w'], np.float32).T.copy(),  # (2,128)
    }
    for nm in ('kw1', 'vw1'):
        w = np.asarray(inputs[nm], np.float32)
        base[f'{nm}_0'] = w[0:128].copy()
        base[f'{nm}_1'] = w[128:256].copy()
        base[f'{nm}_2'] = w[256:260].copy()
    for nm in ('kw2', 'vw2', 'qw1', 'qw2'):
        w = np.asarray(inputs[nm], np.float32)
        base[f'{nm}_0'] = w[0:128].copy()
        base[f'{nm}_1'] = w[128:256].copy()
    maps = []
    for c in range(NCORE):
        b, k = c // 4, c % 4
        m = dict(base)
        m['inp_pad'] = inp_pad[b].reshape(3, 66 * 66).copy()
        m['coordq'] = coord[b, k * QPC:(k + 1) * QPC].copy()
        m['cellq'] = cell[b, k * QPC:(k + 1) * QPC].copy()
        m['cell00'] = cell[b, 0:1, :].copy()
        maps.append(m)
    return maps


def kernel(**inputs):
    from concourse.bass_utils import run_bass_kernel_spmd
    if 'nc' not in _cache:
        _cache['nc'] = _build()
    nc = _cache['nc']
    in_maps = _prep_inputs(inputs)
    res = run_bass_kernel_spmd(nc, in_maps, core_ids=list(range(NCORE)))
    out = np.zeros((B, Q, 3), np.float32)
    for c in range(NCORE):
        b, k = c // 4, c % 4
        out[b, k * QPC:(k + 1) * QPC] = res.results[c]['outq'][:, :3]
    return out

